# revision 8
# baseline (speedup 1.0000x reference)
"""DiffTransformer layer on 8 TRN2 NeuronCores.

Sharding: core c = (batch b=c//2, head-group g=c%2). Each core computes
q/k/v projections + differential attention for its 8 heads of its batch
(all in transposed [feature, seq] layout), a partial out-projection over
its 512 attention channels, then a pair ReduceScatter ([0,1],[2,3],...)
sums the two head-groups' partials and hands each core a 512-seq shard,
on which it runs the full FFN + residual + final RMSNorm.

Numerics: matmuls in float32r (TF32-like, ~11-bit mantissa, full rate on
the PE at moving-dim>=256) with fp32 PSUM accumulation. Softmax without
max-subtraction (scores bounded ~|4|), causal masking as an additive
-40 band folded into the score PSUM via an identity matmul, softmax
denominators via an appended ones-column on v (Z rides along in the PV
matmul), division deferred and folded into the subln RMS via scale
invariance (eps corrected by (Z1*Z2)^2). RoPE applied on PSUM eviction
via a stream-shuffle swap with host-permuted (evens-then-odds) q/k
weight rows.
"""
import os
import sys
import numpy as np

for _p in ("/opt/trn_rl_repo", "/root/.axon_site/_ro/trn_rl_repo"):
    if os.path.isdir(_p) and _p not in sys.path:
        sys.path.append(_p)

B, S, D, H, HD, FF = 4, 1024, 1024, 16, 32, 4096
NCORES = 8
LAMBDA_INIT = 0.8 - 0.6 * float(np.exp(-0.3 * 12))
EPS = 1e-5
SCALE = float(HD) ** -0.5

SWAP16 = [((i + 16) % 32) for i in range(32)]

LAST_RESULT = None  # BassKernelResults of the most recent run (for test.py)
_PROGRAM = {}


def _kts(qc):
    # (k-tile index, diag-band offset or None=full) for a 512-wide q chunk
    if qc == 0:
        return [(0, 0), (1, 128), (2, 256), (3, 384)]
    return [(0, None), (1, None), (2, None), (3, None),
            (4, 0), (5, 128), (6, 256), (7, 384)]


def _build_program():
    import concourse.bacc as bacc
    import concourse.mybir as mybir
    from concourse import tile
    from contextlib import ExitStack

    dt = mybir.dt
    f32, f32r = dt.float32, dt.float32r
    Alu = mybir.AluOpType
    Act = mybir.ActivationFunctionType

    nc = bacc.Bacc("TRN2", target_bir_lowering=False, debug=False,
                   num_devices=NCORES)

    P = 128
    xT = nc.declare_dram_parameter("xT", [D, S], f32r, isOutput=False)
    wqT = nc.declare_dram_parameter("wqT", [D, 512], f32r, isOutput=False)
    wkT = nc.declare_dram_parameter("wkT", [D, 512], f32r, isOutput=False)
    wvT = nc.declare_dram_parameter("wvT", [D, 512], f32r, isOutput=False)
    woT = nc.declare_dram_parameter("woT", [512, D], f32r, isOutput=False)
    w1s = nc.declare_dram_parameter("w1s", [32, P, 1024], f32r, isOutput=False)
    w2T = nc.declare_dram_parameter("w2T", [FF, D], f32r, isOutput=False)
    b1c = nc.declare_dram_parameter("b1c", [P, 32], f32, isOutput=False)
    b2c = nc.declare_dram_parameter("b2c", [P, 8], f32, isOutput=False)
    rmswc = nc.declare_dram_parameter("rmswc", [P, 8], f32, isOutput=False)
    lam128 = nc.declare_dram_parameter("lam128", [P, 1], f32, isOutput=False)
    cosT = nc.declare_dram_parameter("cosT", [P, S], f32, isOutput=False)
    sinS = nc.declare_dram_parameter("sinS", [P, S], f32, isOutput=False)
    ident = nc.declare_dram_parameter("ident", [P, P], f32r, isOutput=False)
    maskneg = nc.declare_dram_parameter("maskneg", [P, P], f32r, isOutput=False)
    hsel = nc.declare_dram_parameter("hsel", [P, P], f32r, isOutput=False)
    bsel = nc.declare_dram_parameter("bsel", [P, P], f32r, isOutput=False)
    outT = nc.declare_dram_parameter("outT", [D, 512], f32, isOutput=True)
    debug = bool(int(os.environ.get("KERNEL_DEBUG", "0")))
    if debug:
        dbg_q = nc.declare_dram_parameter("dbg_q", [512, S], f32, isOutput=True)
        dbg_k = nc.declare_dram_parameter("dbg_k", [512, S], f32, isOutput=True)
        dbg_va = nc.declare_dram_parameter("dbg_va", [8 * P, 520], f32, isOutput=True)
        dbg_at = nc.declare_dram_parameter("dbg_at", [512, S], f32, isOutput=True)
        dbg_rs = nc.declare_dram_parameter("dbg_rs", [D, 512], f32, isOutput=True)
        dbg_po = nc.declare_dram_parameter("dbg_po", [2 * D, 512], f32, isOutput=True)

    with tile.TileContext(nc) as tc:
        with (
            tc.tile_pool(name="consts", bufs=1) as consts,
            tc.tile_pool(name="dram", bufs=1, space="DRAM") as dram,
        ):
            # ---- constants -------------------------------------------
            cos_sb = consts.tile([P, S], f32, tag="cos")
            sin_sb = consts.tile([P, S], f32, tag="sin")
            id_sb = consts.tile([P, P], f32r, tag="idn")
            mn_sb = consts.tile([P, P], f32r, tag="mn")
            hs_sb = consts.tile([P, P], f32r, tag="hs")
            bs_sb = consts.tile([P, P], f32r, tag="bs")
            lam_sb = consts.tile([P, 1], f32, tag="lam")
            b1_sb = consts.tile([P, 32], f32, tag="b1")
            b2_sb = consts.tile([P, 8], f32, tag="b2")
            rw_sb = consts.tile([P, 8], f32, tag="rw")
            wo_sb = [consts.tile([P, D], f32r, tag=f"wo{i}", name=f"wo{i}")
                     for i in range(4)]
            nc.sync.dma_start(cos_sb[:], cosT[:])
            nc.sync.dma_start(sin_sb[:], sinS[:])
            nc.sync.dma_start(id_sb[:], ident[:])
            nc.sync.dma_start(mn_sb[:], maskneg[:])
            nc.sync.dma_start(hs_sb[:], hsel[:])
            nc.sync.dma_start(bs_sb[:], bsel[:])
            nc.sync.dma_start(lam_sb[:], lam128[:])
            nc.sync.dma_start(b1_sb[:], b1c[:])
            nc.sync.dma_start(b2_sb[:], b2c[:])
            nc.sync.dma_start(rw_sb[:], rmswc[:])
            for i in range(4):
                nc.sync.dma_start(wo_sb[i][:], woT[i * P:(i + 1) * P, :])
            ones_f32 = consts.tile([P, 8], f32, tag="onesf")
            nc.vector.memset(ones_f32[:], 1.0)
            eps_sb = consts.tile([1, 1], f32, tag="eps")
            nc.vector.memset(eps_sb[:], EPS)
            ones_r = consts.tile([P, 1], f32r, tag="onesr")
            nc.vector.tensor_copy(ones_r[:], ones_f32[:, 0:1])

            _stk = ExitStack()
            qkpool = _stk.enter_context(tc.tile_pool(name="qk", bufs=1))
            vapool = _stk.enter_context(tc.tile_pool(name="vaug", bufs=1))
            attnpool = _stk.enter_context(tc.tile_pool(name="attn", bufs=1))

            qT = [qkpool.tile([P, S], f32r, tag=f"qT{i}", name=f"qT{i}")
                  for i in range(4)]
            kT = [qkpool.tile([P, S], f32r, tag=f"kT{i}", name=f"kT{i}")
                  for i in range(4)]
            va = [vapool.tile([P, 8 * 65], f32r, tag=f"va{i}", name=f"va{i}")
                  for i in range(8)]
            attnT = [attnpool.tile([P, S], f32r, tag=f"at{i}", name=f"at{i}")
                     for i in range(4)]

            po_dram = dram.tile([2, D, 512], f32)
            rs_dram = dram.tile([D, 512], f32)

            # ---- phase 1+2: load xT / weights, project v then q,k ----
            with (
                tc.tile_pool(name="xw", bufs=1) as xw,
                tc.tile_pool(name="proj_ps", bufs=6, space="PSUM") as pps,
                tc.tile_pool(name="rtmp", bufs=4) as rtmp,
            ):
                xt = [xw.tile([P, S], f32r, tag=f"x{i}", name=f"x{i}")
                      for i in range(8)]
                wq_sb = [xw.tile([P, 512], f32r, tag=f"wq{i}", name=f"wqs{i}")
                         for i in range(8)]
                wk_sb = [xw.tile([P, 512], f32r, tag=f"wk{i}", name=f"wks{i}")
                         for i in range(8)]
                wv_sb = [xw.tile([P, 512], f32r, tag=f"wv{i}", name=f"wvs{i}")
                         for i in range(8)]
                for i in range(8):
                    nc.sync.dma_start(xt[i][:], xT[i * P:(i + 1) * P, :])
                    nc.sync.dma_start(wv_sb[i][:], wvT[i * P:(i + 1) * P, :])
                    nc.sync.dma_start(wq_sb[i][:], wqT[i * P:(i + 1) * P, :])
                    nc.sync.dma_start(wk_sb[i][:], wkT[i * P:(i + 1) * P, :])

                # v direct to [seq, chan], evicted strided into v_aug
                for st in range(8):
                    ps = pps.tile([P, 512], f32, tag="ps")
                    for kd in range(8):
                        nc.tensor.matmul(
                            ps[:],
                            lhsT=xt[kd][:, st * P:(st + 1) * P],
                            rhs=wv_sb[kd][:],
                            start=(kd == 0), stop=(kd == 7))
                    va3 = va[st][:].rearrange("p (h e) -> p h e", h=8, e=65)
                    nc.any.tensor_copy(
                        va3[:, :, 0:64],
                        ps[:].rearrange("p (h e) -> p h e", h=8, e=64))
                    nc.any.tensor_copy(
                        va3[:, :, 64:65],
                        ones_f32[:].rearrange("p (h o) -> p h o", o=1))

                # q,k with fused RoPE on eviction; per chan-tile so that
                # attention on part-tile pt can start early
                for mt in range(4):
                    for wsb, dstT in ((wq_sb, qT), (wk_sb, kT)):
                        for nch in range(2):
                            n0 = nch * 512
                            ps = pps.tile([P, 512], f32, tag="ps")
                            for kd in range(8):
                                nc.tensor.matmul(
                                    ps[:],
                                    lhsT=wsb[kd][:, mt * P:(mt + 1) * P],
                                    rhs=xt[kd][:, n0:n0 + 512],
                                    start=(kd == 0), stop=(kd == 7))
                            dst = dstT[mt][:, n0:n0 + 512]
                            nc.vector.tensor_tensor(
                                dst, ps[:], cos_sb[:, n0:n0 + 512], Alu.mult)
                            tmp = rtmp.tile([P, 512], f32, tag="rt")
                            nc.vector.stream_shuffle(tmp[:], ps[:], SWAP16)
                            nc.vector.tensor_tensor(
                                tmp[:], tmp[:], sin_sb[:, n0:n0 + 512], Alu.mult)
                            nc.vector.tensor_tensor(dst, dst, tmp[:], Alu.add)

            # ---- phase 3: differential attention + out-projection ----
            with (
                tc.tile_pool(name="pv_ps", bufs=4, space="PSUM") as pvp,
                tc.tile_pool(name="st_ps", bufs=4, space="PSUM") as stp,
                tc.tile_pool(name="epool", bufs=6) as epool,
                tc.tile_pool(name="post", bufs=2) as post,
                tc.tile_pool(name="small", bufs=3) as small,
                tc.tile_pool(name="po_sb", bufs=4) as posb,
            ):
                for qc in range(2):
                    q0 = qc * 512
                    kts = _kts(qc)
                    last_kt = kts[-1][0]
                    for pt in range(4):
                        pvs = [pvp.tile([P, 512], f32, tag="pv", name="pv")
                               for _ in range(4)]
                        for kt, off in kts:
                            j0 = 0 if off is None else off
                            sts = []
                            for gq in range(4):
                                st_ps = stp.tile([P, 512], f32, tag="st",
                                                 name="st")
                                if off is not None:
                                    nc.tensor.matmul(
                                        st_ps[:, j0:j0 + P],
                                        lhsT=id_sb[:], rhs=mn_sb[:],
                                        start=True, stop=False)
                                nc.tensor.matmul(
                                    st_ps[:, j0:],
                                    lhsT=kT[pt][gq * 32:(gq + 1) * 32,
                                                kt * P:(kt + 1) * P],
                                    rhs=qT[pt][gq * 32:(gq + 1) * 32,
                                               q0 + j0:q0 + 512],
                                    start=(off is None), stop=True,
                                    tile_position=(gq * 32, 0))
                                sts.append(st_ps)
                            for gq in range(4):
                                e = epool.tile([P, 512], f32r, tag="e",
                                               name="e")
                                nc.scalar.activation(
                                    e[:, j0:], sts[gq][:, j0:], Act.Exp,
                                    scale=SCALE)
                                h_loc = (pt * P + gq * 32) // 64
                                nc.tensor.matmul(
                                    pvs[gq][0:65, j0:],
                                    lhsT=va[kt][:, h_loc * 65:(h_loc + 1) * 65],
                                    rhs=e[:, j0:],
                                    start=(kt == 0), stop=(kt == last_kt))

                        # evict PV psum fast: stack the two heads' A1/A2
                        # and their Z rows (rows 0 and 64 of zrow tiles)
                        a1 = post.tile([P, 512], f32, tag="a1")
                        a2 = post.tile([P, 512], f32, tag="a2")
                        zr1 = post.tile([P, 512], f32r, tag="zr1")
                        zr2 = post.tile([P, 512], f32r, tag="zr2")
                        for hp in range(2):
                            nc.vector.tensor_copy(a1[hp * 64:(hp + 1) * 64, :],
                                                  pvs[2 * hp][0:64, :])
                            nc.vector.tensor_copy(a2[hp * 64:(hp + 1) * 64, :],
                                                  pvs[2 * hp + 1][0:64, :])
                            nc.vector.tensor_copy(zr1[hp * 64:hp * 64 + 1, :],
                                                  pvs[2 * hp][64:65, :])
                            nc.vector.tensor_copy(zr2[hp * 64:hp * 64 + 1, :],
                                                  pvs[2 * hp + 1][64:65, :])
                        # PE broadcast of Z rows to the two 64-row halves
                        bc1 = stp.tile([P, 512], f32, tag="st", name="bc1")
                        nc.tensor.matmul(bc1[:], lhsT=bs_sb[:], rhs=zr1[:],
                                         start=True, stop=True)
                        bc2 = stp.tile([P, 512], f32, tag="st", name="bc2")
                        nc.tensor.matmul(bc2[:], lhsT=bs_sb[:], rhs=zr2[:],
                                         start=True, stop=True)
                        t1 = post.tile([P, 512], f32, tag="t1")
                        t2 = post.tile([P, 512], f32, tag="t2")
                        nc.vector.tensor_tensor(t1[:], a1[:], bc2[:], Alu.mult)
                        nc.vector.tensor_tensor(t2[:], a2[:], bc1[:], Alu.mult)
                        negw = post.tile([P, 512], f32, tag="negw")
                        nc.vector.scalar_tensor_tensor(
                            negw[:], in0=t2[:], scalar=lam_sb[:, 0:1],
                            in1=t1[:], op0=Alu.mult, op1=Alu.subtract)
                        sq = post.tile([P, 512], f32r, tag="sq")
                        nc.vector.tensor_tensor(sq[:], negw[:], negw[:], Alu.mult)
                        ms_ps = stp.tile([P, 512], f32, tag="st", name="msps")
                        nc.tensor.matmul(ms_ps[:], lhsT=hs_sb[:], rhs=sq[:],
                                         start=True, stop=True)
                        # mss rows 0/64 = ms/64 + EPS*(Z1*Z2)^2 per head
                        mss = post.tile([P, 512], f32r, tag="mss")
                        nc.vector.tensor_copy(mss[:], ms_ps[:])
                        for hp in range(2):
                            r0 = hp * 64
                            z12 = small.tile([1, 512], f32, tag="z12")
                            nc.vector.tensor_tensor(
                                z12[:], zr1[r0:r0 + 1, :], zr2[r0:r0 + 1, :],
                                Alu.mult)
                            msb = small.tile([1, 512], f32, tag="msb")
                            nc.vector.scalar_tensor_tensor(
                                msb[:], in0=z12[:], scalar=EPS, in1=z12[:],
                                op0=Alu.mult, op1=Alu.mult)
                            nc.vector.scalar_tensor_tensor(
                                mss[r0:r0 + 1, :], in0=ms_ps[r0:r0 + 1, :],
                                scalar=1.0 / 64.0, in1=msb[:],
                                op0=Alu.mult, op1=Alu.add)
                        bcm = stp.tile([P, 512], f32, tag="st", name="bcm")
                        nc.tensor.matmul(bcm[:], lhsT=bs_sb[:], rhs=mss[:],
                                         start=True, stop=True)
                        srt = post.tile([P, 512], f32, tag="srt")
                        nc.scalar.activation(srt[:], bcm[:], Act.Sqrt)
                        rstd = post.tile([P, 512], f32, tag="rstd")
                        nc.vector.reciprocal_approx_fast(rstd[:], srt[:])
                        nc.vector.scalar_tensor_tensor(
                            attnT[pt][:, q0:q0 + 512],
                            in0=negw[:], scalar=-(1.0 - LAMBDA_INIT),
                            in1=rstd[:], op0=Alu.mult, op1=Alu.mult)

                    # out-projection for this seq half (overlaps next qc)
                    for mo in range(8):
                        ps = stp.tile([P, 512], f32, tag="st", name="wops")
                        for kc in range(4):
                            nc.tensor.matmul(
                                ps[:],
                                lhsT=wo_sb[kc][:, mo * P:(mo + 1) * P],
                                rhs=attnT[kc][:, q0:q0 + 512],
                                start=(kc == 0), stop=(kc == 3))
                        po = posb.tile([P, 512], f32, tag="po")
                        nc.any.tensor_copy(po[:], ps[:])
                        nc.sync.dma_start(
                            po_dram[qc, mo * P:(mo + 1) * P, :], po[:])

            if debug:
                with tc.tile_pool(name="dbgp", bufs=2) as dbgp:
                    for i in range(4):
                        dq = dbgp.tile([P, S], f32, tag="dq", name="dq")
                        nc.vector.tensor_copy(dq[:], qT[i][:])
                        nc.sync.dma_start(dbg_q[i * P:(i + 1) * P, :], dq[:])
                        dk = dbgp.tile([P, S], f32, tag="dk", name="dk")
                        nc.vector.tensor_copy(dk[:], kT[i][:])
                        nc.sync.dma_start(dbg_k[i * P:(i + 1) * P, :], dk[:])
                        da = dbgp.tile([P, S], f32, tag="da", name="da")
                        nc.vector.tensor_copy(da[:], attnT[i][:])
                        nc.sync.dma_start(dbg_at[i * P:(i + 1) * P, :], da[:])
                    for i in range(8):
                        dv = dbgp.tile([P, 520], f32, tag="dv", name="dv")
                        nc.vector.tensor_copy(dv[:], va[i][:])
                        nc.sync.dma_start(dbg_va[i * P:(i + 1) * P, :], dv[:])

            _stk.close()

            nc.gpsimd.collective_compute(
                "ReduceScatter",
                mybir.AluOpType.add,
                replica_groups=[[0, 1], [2, 3], [4, 5], [6, 7]],
                ins=[po_dram.opt()],
                outs=[rs_dram.opt()],
            )

            # ---- phase 5: FFN + residual + final RMS on seq shard ----
            with (
                tc.tile_pool(name="aT", bufs=1) as atp,
                tc.tile_pool(name="astage", bufs=2) as astage,
                tc.tile_pool(name="h1", bufs=1) as h1p,
                tc.tile_pool(name="w1p", bufs=6) as w1p,
                tc.tile_pool(name="w2p", bufs=3) as w2p,
                tc.tile_pool(name="yT", bufs=1) as ytp,
                tc.tile_pool(name="fin", bufs=2) as finp,
                tc.tile_pool(name="sm2", bufs=1) as sm2,
            ):
                aTr = [atp.tile([P, 512], f32r, tag=f"ar{i}", name=f"ar{i}")
                       for i in range(8)]
                for i in range(8):
                    stg = astage.tile([P, 512], f32, tag="stg")
                    nc.sync.dma_start(stg[:], rs_dram[i * P:(i + 1) * P, :])
                    nc.vector.tensor_copy(aTr[i][:], stg[:])
                    if debug:
                        nc.sync.dma_start(dbg_rs[i * P:(i + 1) * P, :], stg[:])
                if debug:
                    nc.sync.dma_start(dbg_po[0:D, :], po_dram[0])
                    nc.sync.dma_start(dbg_po[D:2 * D, :], po_dram[1])

                h1 = [h1p.tile([P, 512], f32r, tag=f"h1_{i}", name=f"h1_{i}")
                      for i in range(32)]
                with tc.tile_pool(name="h1_ps", bufs=4, space="PSUM") as h1ps:
                    for mf in range(32):
                        wt = w1p.tile([P, 1024], f32r, tag="w1t", name="w1t")
                        nc.sync.dma_start(wt[:], w1s[mf, :, :])
                        ps = h1ps.tile([P, 512], f32, tag="h1ps", name="h1ps")
                        for kd in range(8):
                            nc.tensor.matmul(
                                ps[:], lhsT=wt[:, kd * P:(kd + 1) * P],
                                rhs=aTr[kd][:], start=(kd == 0), stop=(kd == 7))
                        nc.scalar.activation(h1[mf][:], ps[:], Act.Relu,
                                             bias=b1_sb[:, mf:mf + 1])

                # h2: 8 persistent PSUM accumulators, stream w2 tiles
                with tc.tile_pool(name="h2_ps", bufs=1, space="PSUM") as h2ps:
                    ps8 = [h2ps.tile([P, 512], f32, tag=f"h2_{mo}",
                                     name=f"h2_{mo}") for mo in range(8)]
                    for kf in range(32):
                        wt2 = w2p.tile([P, 1024], f32r, tag="w2t", name="w2t")
                        nc.sync.dma_start(wt2[:], w2T[kf * P:(kf + 1) * P, :])
                        for mo in range(8):
                            nc.tensor.matmul(
                                ps8[mo][:], lhsT=wt2[:, mo * P:(mo + 1) * P],
                                rhs=h1[kf][:], start=(kf == 0), stop=(kf == 31))
                    yt = [ytp.tile([P, 512], f32, tag=f"y{i}", name=f"y{i}")
                          for i in range(8)]
                    for mo in range(8):
                        nc.vector.scalar_tensor_tensor(
                            yt[mo][:], in0=ps8[mo][:],
                            scalar=b2_sb[:, mo:mo + 1], in1=aTr[mo][:],
                            op0=Alu.add, op1=Alu.add)

                # final RMS over D (partition dim across the 8 tiles)
                with tc.tile_pool(name="rms_ps", bufs=1, space="PSUM") as rmsps:
                    ms_ps = rmsps.tile([P, 512], f32, tag="rmsps", name="rmsps")
                    for mo in range(8):
                        sq = finp.tile([P, 512], f32r, tag="fsq", name="fsq")
                        nc.vector.tensor_tensor(sq[:], yt[mo][:], yt[mo][:],
                                                Alu.mult)
                        nc.tensor.matmul(ms_ps[0:1, :], lhsT=ones_r[:],
                                         rhs=sq[:], start=(mo == 0),
                                         stop=(mo == 7))
                    srt = sm2.tile([1, 512], f32, tag="fsrt")
                    nc.scalar.activation(srt[:], ms_ps[0:1, :], Act.Sqrt,
                                         scale=1.0 / 1024.0, bias=eps_sb[:])
                    rstd = sm2.tile([1, 512], f32, tag="frstd")
                    nc.vector.reciprocal_approx_fast(rstd[:], srt[:])
                    bcr = sm2.tile([P, 512], f32, tag="fbcr")
                    nc.gpsimd.partition_broadcast(bcr[:], rstd[:])
                    for mo in range(8):
                        ot = finp.tile([P, 512], f32, tag="fot", name="fot")
                        nc.vector.scalar_tensor_tensor(
                            ot[:], in0=yt[mo][:], scalar=rw_sb[:, mo:mo + 1],
                            in1=bcr[:], op0=Alu.mult, op1=Alu.mult)
                        nc.sync.dma_start(outT[mo * P:(mo + 1) * P, :], ot[:])

    nc.compile()
    return nc


def _host_prep(inputs):
    x = np.ascontiguousarray(np.asarray(inputs["x"], dtype=np.float32))
    Wq = np.asarray(inputs["Wq"], dtype=np.float32)
    Wk = np.asarray(inputs["Wk"], dtype=np.float32)
    Wv = np.asarray(inputs["Wv"], dtype=np.float32)
    Wo = np.asarray(inputs["Wo"], dtype=np.float32)
    W1 = np.asarray(inputs["W1"], dtype=np.float32)
    b1 = np.asarray(inputs["b1"], dtype=np.float32)
    W2 = np.asarray(inputs["W2"], dtype=np.float32)
    b2 = np.asarray(inputs["b2"], dtype=np.float32)
    rmsw = np.asarray(inputs["rms_weight"], dtype=np.float32)
    lam = float(np.exp(np.dot(np.asarray(inputs["lambda_q1"], np.float64),
                              np.asarray(inputs["lambda_k1"], np.float64)))
                - np.exp(np.dot(np.asarray(inputs["lambda_q2"], np.float64),
                                np.asarray(inputs["lambda_k2"], np.float64)))
                + LAMBDA_INIT)

    half = HD // 2
    cos16 = sin16 = None
    try:
        import jax.numpy as jnp
        freqs = 1.0 / (10000.0 ** (jnp.arange(half, dtype=jnp.float32) / half))
        ang = jnp.arange(S, dtype=jnp.float32)[:, None] * freqs[None, :]
        cos16 = np.asarray(jnp.cos(ang)).T.astype(np.float32)
        sin16 = np.asarray(jnp.sin(ang)).T.astype(np.float32)
    except Exception:
        pass
    if cos16 is None:
        freqs = (1.0 / (10000.0 ** (np.arange(half, dtype=np.float32)
                                    / np.float32(half)))).astype(np.float32)
        ang = (np.arange(S, dtype=np.float32)[:, None] * freqs[None, :])
        cos16 = np.cos(ang.astype(np.float32)).T.astype(np.float32)
        sin16 = np.sin(ang.astype(np.float32)).T.astype(np.float32)

    cosT = np.ascontiguousarray(np.tile(np.concatenate([cos16, cos16], 0), (4, 1)))
    sinS = np.ascontiguousarray(
        np.tile(np.concatenate([-sin16, sin16], 0), (4, 1))).astype(np.float32)
    perm32 = np.concatenate([np.arange(0, 32, 2), np.arange(1, 32, 2)])

    idn = np.eye(128, dtype=np.float32)
    mneg = ((np.arange(128)[:, None] > np.arange(128)[None, :])
            .astype(np.float32) * -240.0)
    hsel = np.zeros((128, 128), np.float32)
    hsel[0:64, 0] = 1.0
    hsel[64:128, 64] = 1.0
    bsel = np.zeros((128, 128), np.float32)
    bsel[0, 0:64] = 1.0
    bsel[64, 64:128] = 1.0

    b1c = np.ascontiguousarray(b1.reshape(32, 128).T)
    b2c = np.ascontiguousarray(b2.reshape(8, 128).T)
    rmswc = np.ascontiguousarray(rmsw.reshape(8, 128).T)
    lam128 = np.full((128, 1), lam, np.float32)
    # w1s[mf][p, kd*128+j] = W1.T[kd*128+p, mf*128+j]
    w1s = np.ascontiguousarray(
        W1.T.reshape(8, 128, 32, 128).transpose(2, 1, 0, 3).reshape(32, 128, 1024))
    w2T = np.ascontiguousarray(W2.T)

    in_maps = []
    for c in range(NCORES):
        b, g = c // 2, c % 2
        chans = np.arange(g * 512, (g + 1) * 512)
        permed = np.concatenate(
            [c0 * 32 + perm32 for c0 in range(g * 16, (g + 1) * 16)])
        in_maps.append({
            "xT": np.ascontiguousarray(x[b].T),
            "wqT": np.ascontiguousarray(Wq[permed, :].T),
            "wkT": np.ascontiguousarray(Wk[permed, :].T),
            "wvT": np.ascontiguousarray(Wv[chans, :].T),
            "woT": np.ascontiguousarray(Wo[:, chans].T),
            "w1s": w1s, "w2T": w2T,
            "b1c": b1c, "b2c": b2c, "rmswc": rmswc, "lam128": lam128,
            "cosT": cosT, "sinS": sinS,
            "ident": idn, "maskneg": mneg, "hsel": hsel, "bsel": bsel,
        })
    return in_maps


def kernel(**inputs):
    global LAST_RESULT
    from concourse.bass_utils import run_bass_kernel_spmd

    key = "nc_dbg" if os.environ.get("KERNEL_DEBUG", "0") == "1" else "nc"
    if key not in _PROGRAM:
        _PROGRAM[key] = _build_program()
    nc = _PROGRAM[key]

    in_maps = _host_prep(inputs)
    trace = bool(int(os.environ.get("KERNEL_TRACE", "0")))
    res = run_bass_kernel_spmd(nc, in_maps, list(range(NCORES)), trace=trace)
    LAST_RESULT = res

    out = np.empty((B, S, D), np.float32)
    for c in range(NCORES):
        b, g = c // 2, c % 2
        out[b, g * 512:(g + 1) * 512, :] = res.results[c]["outT"].T
    return out


# revision 9
# speedup vs baseline: 1.0576x; 1.0576x over previous
"""DiffTransformer layer on 8 TRN2 NeuronCores.

Sharding: core c = (batch b=c//2, head-group g=c%2). Each core computes
q/k/v projections + differential attention for its 8 heads of its batch
(all in transposed [feature, seq] layout), a partial out-projection over
its 512 attention channels, then a pair ReduceScatter ([0,1],[2,3],...)
sums the two head-groups' partials and hands each core a 512-seq shard,
on which it runs the full FFN + residual + final RMSNorm.

Numerics: matmuls in float32r (TF32-like, ~11-bit mantissa, full rate on
the PE at moving-dim>=256) with fp32 PSUM accumulation. Softmax without
max-subtraction (scores bounded ~|4|), causal masking as an additive
-40 band folded into the score PSUM via an identity matmul, softmax
denominators via an appended ones-column on v (Z rides along in the PV
matmul), division deferred and folded into the subln RMS via scale
invariance (eps corrected by (Z1*Z2)^2). RoPE applied on PSUM eviction
via a stream-shuffle swap with host-permuted (evens-then-odds) q/k
weight rows.
"""
import os
import sys
import numpy as np

for _p in ("/opt/trn_rl_repo", "/root/.axon_site/_ro/trn_rl_repo"):
    if os.path.isdir(_p) and _p not in sys.path:
        sys.path.append(_p)

B, S, D, H, HD, FF = 4, 1024, 1024, 16, 32, 4096
NCORES = 8
LAMBDA_INIT = 0.8 - 0.6 * float(np.exp(-0.3 * 12))
EPS = 1e-5
SCALE = float(HD) ** -0.5

SWAP16 = [((i + 16) % 32) for i in range(32)]

LAST_RESULT = None  # BassKernelResults of the most recent run (for test.py)
_PROGRAM = {}


def _kts(qc):
    # (k-tile index, diag-band offset or None=full) for a 512-wide q chunk
    if qc == 0:
        return [(0, 0), (1, 128), (2, 256), (3, 384)]
    return [(0, None), (1, None), (2, None), (3, None),
            (4, 0), (5, 128), (6, 256), (7, 384)]


def _build_program():
    import concourse.bacc as bacc
    import concourse.mybir as mybir
    from concourse import tile
    from contextlib import ExitStack

    dt = mybir.dt
    f32, f32r = dt.float32, dt.float32r
    Alu = mybir.AluOpType
    Act = mybir.ActivationFunctionType

    nc = bacc.Bacc("TRN2", target_bir_lowering=False, debug=False,
                   num_devices=NCORES)

    P = 128
    xT = nc.declare_dram_parameter("xT", [D, S], f32r, isOutput=False)
    wqT = nc.declare_dram_parameter("wqT", [D, 512], f32r, isOutput=False)
    wkT = nc.declare_dram_parameter("wkT", [D, 512], f32r, isOutput=False)
    wvT = nc.declare_dram_parameter("wvT", [D, 512], f32r, isOutput=False)
    woT = nc.declare_dram_parameter("woT", [512, D], f32r, isOutput=False)
    w1s = nc.declare_dram_parameter("w1s", [32, P, 1024], f32r, isOutput=False)
    w2T = nc.declare_dram_parameter("w2T", [FF, D], f32r, isOutput=False)
    b1c = nc.declare_dram_parameter("b1c", [P, 32], f32, isOutput=False)
    b2c = nc.declare_dram_parameter("b2c", [P, 8], f32, isOutput=False)
    rmswc = nc.declare_dram_parameter("rmswc", [P, 8], f32, isOutput=False)
    lam128 = nc.declare_dram_parameter("lam128", [P, 1], f32, isOutput=False)
    cosT = nc.declare_dram_parameter("cosT", [P, S], f32, isOutput=False)
    sinS = nc.declare_dram_parameter("sinS", [P, S], f32, isOutput=False)
    hsel = nc.declare_dram_parameter("hsel", [P, P], f32r, isOutput=False)
    mdiag = nc.declare_dram_parameter("mdiag", [P, P], f32, isOutput=False)
    outT = nc.declare_dram_parameter("outT", [D, 512], f32, isOutput=True)
    debug = bool(int(os.environ.get("KERNEL_DEBUG", "0")))
    if debug:
        dbg_q = nc.declare_dram_parameter("dbg_q", [512, S], f32, isOutput=True)
        dbg_k = nc.declare_dram_parameter("dbg_k", [512, S], f32, isOutput=True)
        dbg_va = nc.declare_dram_parameter("dbg_va", [8 * P, 520], f32, isOutput=True)
        dbg_at = nc.declare_dram_parameter("dbg_at", [512, S], f32, isOutput=True)
        dbg_rs = nc.declare_dram_parameter("dbg_rs", [D, 512], f32, isOutput=True)
        dbg_po = nc.declare_dram_parameter("dbg_po", [2 * D, 512], f32, isOutput=True)

    with tile.TileContext(nc) as tc:
        with (
            tc.tile_pool(name="consts", bufs=1) as consts,
            tc.tile_pool(name="dram", bufs=1, space="DRAM") as dram,
        ):
            # ---- constants -------------------------------------------
            cos_sb = consts.tile([P, S], f32, tag="cos")
            sin_sb = consts.tile([P, S], f32, tag="sin")
            hs_sb = consts.tile([P, P], f32r, tag="hs")
            md_sb = consts.tile([P, P], f32, tag="md")
            lam_sb = consts.tile([P, 1], f32, tag="lam")
            b1_sb = consts.tile([P, 32], f32, tag="b1")
            b2_sb = consts.tile([P, 8], f32, tag="b2")
            rw_sb = consts.tile([P, 8], f32, tag="rw")
            wo_sb = [consts.tile([P, D], f32r, tag=f"wo{i}", name=f"wo{i}")
                     for i in range(4)]
            nc.sync.dma_start(cos_sb[:], cosT[:])
            nc.sync.dma_start(sin_sb[:], sinS[:])
            nc.sync.dma_start(hs_sb[:], hsel[:])
            nc.sync.dma_start(md_sb[:], mdiag[:])
            nc.sync.dma_start(lam_sb[:], lam128[:])
            nc.sync.dma_start(b1_sb[:], b1c[:])
            nc.sync.dma_start(b2_sb[:], b2c[:])
            nc.sync.dma_start(rw_sb[:], rmswc[:])
            for i in range(4):
                nc.sync.dma_start(wo_sb[i][:], woT[i * P:(i + 1) * P, :])
            ones_f32 = consts.tile([P, 8], f32, tag="onesf")
            nc.vector.memset(ones_f32[:], 1.0)
            eps_sb = consts.tile([1, 1], f32, tag="eps")
            nc.vector.memset(eps_sb[:], EPS)
            ones_r = consts.tile([P, 1], f32r, tag="onesr")
            nc.vector.tensor_copy(ones_r[:], ones_f32[:, 0:1])

            _stk = ExitStack()
            qkpool = _stk.enter_context(tc.tile_pool(name="qk", bufs=1))
            vapool = _stk.enter_context(tc.tile_pool(name="vaug", bufs=1))
            attnpool = _stk.enter_context(tc.tile_pool(name="attn", bufs=1))

            qT = [qkpool.tile([P, S], f32r, tag=f"qT{i}", name=f"qT{i}")
                  for i in range(4)]
            kT = [qkpool.tile([P, S], f32r, tag=f"kT{i}", name=f"kT{i}")
                  for i in range(4)]
            va = [vapool.tile([P, 8 * 65], f32r, tag=f"va{i}", name=f"va{i}")
                  for i in range(8)]
            attnT = [attnpool.tile([P, S], f32r, tag=f"at{i}", name=f"at{i}")
                     for i in range(4)]

            po_dram = dram.tile([2, D, 512], f32)
            rs_dram = dram.tile([D, 512], f32)

            # ---- phase 1+2: load xT / weights, project v then q,k ----
            with (
                tc.tile_pool(name="xw", bufs=1) as xw,
                tc.tile_pool(name="proj_ps", bufs=6, space="PSUM") as pps,
                tc.tile_pool(name="rtmp", bufs=4) as rtmp,
            ):
                xt = [xw.tile([P, S], f32r, tag=f"x{i}", name=f"x{i}")
                      for i in range(8)]
                wq_sb = [xw.tile([P, 512], f32r, tag=f"wq{i}", name=f"wqs{i}")
                         for i in range(8)]
                wk_sb = [xw.tile([P, 512], f32r, tag=f"wk{i}", name=f"wks{i}")
                         for i in range(8)]
                wv_sb = [xw.tile([P, 512], f32r, tag=f"wv{i}", name=f"wvs{i}")
                         for i in range(8)]
                for i in range(8):
                    nc.sync.dma_start(xt[i][:], xT[i * P:(i + 1) * P, :])
                    nc.sync.dma_start(wv_sb[i][:], wvT[i * P:(i + 1) * P, :])
                    nc.sync.dma_start(wq_sb[i][:], wqT[i * P:(i + 1) * P, :])
                    nc.sync.dma_start(wk_sb[i][:], wkT[i * P:(i + 1) * P, :])

                # v direct to [seq, chan], evicted strided into v_aug
                for st in range(8):
                    ps = pps.tile([P, 512], f32, tag="ps")
                    for kd in range(8):
                        nc.tensor.matmul(
                            ps[:],
                            lhsT=xt[kd][:, st * P:(st + 1) * P],
                            rhs=wv_sb[kd][:],
                            start=(kd == 0), stop=(kd == 7))
                    va3 = va[st][:].rearrange("p (h e) -> p h e", h=8, e=65)
                    nc.any.tensor_copy(
                        va3[:, :, 0:64],
                        ps[:].rearrange("p (h e) -> p h e", h=8, e=64))
                    nc.any.tensor_copy(
                        va3[:, :, 64:65],
                        ones_f32[:].rearrange("p (h o) -> p h o", o=1))

                # q,k with fused RoPE on eviction; per chan-tile so that
                # attention on part-tile pt can start early
                for mt in range(4):
                    for wsb, dstT in ((wq_sb, qT), (wk_sb, kT)):
                        for nch in range(2):
                            n0 = nch * 512
                            ps = pps.tile([P, 512], f32, tag="ps")
                            for kd in range(8):
                                nc.tensor.matmul(
                                    ps[:],
                                    lhsT=wsb[kd][:, mt * P:(mt + 1) * P],
                                    rhs=xt[kd][:, n0:n0 + 512],
                                    start=(kd == 0), stop=(kd == 7))
                            dst = dstT[mt][:, n0:n0 + 512]
                            nc.vector.tensor_tensor(
                                dst, ps[:], cos_sb[:, n0:n0 + 512], Alu.mult)
                            tmp = rtmp.tile([P, 512], f32, tag="rt")
                            nc.vector.stream_shuffle(tmp[:], ps[:], SWAP16)
                            nc.vector.tensor_tensor(
                                tmp[:], tmp[:], sin_sb[:, n0:n0 + 512], Alu.mult)
                            nc.vector.tensor_tensor(dst, dst, tmp[:], Alu.add)

            # ---- phase 3: differential attention + out-projection ----
            with (
                tc.tile_pool(name="pv_ps", bufs=4, space="PSUM") as pvp,
                tc.tile_pool(name="st_ps", bufs=4, space="PSUM") as stp,
                tc.tile_pool(name="epool", bufs=6) as epool,
                tc.tile_pool(name="post", bufs=2) as post,
                tc.tile_pool(name="small", bufs=3) as small,
                tc.tile_pool(name="po_sb", bufs=4) as posb,
            ):
                for qc in range(2):
                    q0 = qc * 512
                    kts = _kts(qc)
                    last_kt = kts[-1][0]
                    for pt in range(4):
                        pvs = [pvp.tile([P, 512], f32, tag="pv", name="pv")
                               for _ in range(4)]
                        for kt, off in kts:
                            j0 = 0 if off is None else off
                            sts = []
                            for gq in range(4):
                                st_ps = stp.tile([P, 512], f32, tag="st",
                                                 name="st")
                                nc.tensor.matmul(
                                    st_ps[:, j0:],
                                    lhsT=kT[pt][gq * 32:(gq + 1) * 32,
                                                kt * P:(kt + 1) * P],
                                    rhs=qT[pt][gq * 32:(gq + 1) * 32,
                                               q0 + j0:q0 + 512],
                                    start=True, stop=True,
                                    tile_position=(gq * 32, 0))
                                sts.append(st_ps)
                            for gq in range(4):
                                e = epool.tile([P, 512], f32r, tag="e",
                                               name="e")
                                nc.scalar.activation(
                                    e[:, j0:], sts[gq][:, j0:], Act.Exp,
                                    scale=SCALE)
                                if off is not None:
                                    nc.vector.tensor_tensor(
                                        e[:, j0:j0 + P], e[:, j0:j0 + P],
                                        md_sb[:], Alu.mult)
                                h_loc = (pt * P + gq * 32) // 64
                                nc.tensor.matmul(
                                    pvs[gq][0:65, j0:],
                                    lhsT=va[kt][:, h_loc * 65:(h_loc + 1) * 65],
                                    rhs=e[:, j0:],
                                    start=(kt == 0), stop=(kt == last_kt))

                        # Z rows to base-0 tiles, then gpsimd broadcast
                        zs = [small.tile([1, 512], f32, tag=f"z{i}",
                                         name=f"z{i}") for i in range(4)]
                        for i in range(4):
                            nc.vector.tensor_copy(zs[i][:], pvs[i][64:65, :])
                        bc1 = post.tile([P, 512], f32, tag="bc1")
                        bc2 = post.tile([P, 512], f32, tag="bc2")
                        ubc = post.tile([64, 512], f32, tag="ubc")
                        ubc2 = post.tile([64, 512], f32, tag="ubc2")
                        nc.gpsimd.partition_broadcast(bc1[0:64, :], zs[0][:])
                        nc.gpsimd.partition_broadcast(ubc[:], zs[2][:])
                        nc.vector.tensor_copy(bc1[64:128, :], ubc[:])
                        nc.gpsimd.partition_broadcast(bc2[0:64, :], zs[1][:])
                        nc.gpsimd.partition_broadcast(ubc2[:], zs[3][:])
                        nc.vector.tensor_copy(bc2[64:128, :], ubc2[:])
                        # t1 = A1*Z2bc, t2 = A2*Z1bc straight from PSUM
                        t1 = post.tile([P, 512], f32, tag="t1")
                        t2 = post.tile([P, 512], f32, tag="t2")
                        for hp in range(2):
                            r0 = hp * 64
                            nc.vector.tensor_tensor(
                                t1[r0:r0 + 64, :], pvs[2 * hp][0:64, :],
                                bc2[r0:r0 + 64, :], Alu.mult)
                            nc.vector.tensor_tensor(
                                t2[r0:r0 + 64, :], pvs[2 * hp + 1][0:64, :],
                                bc1[r0:r0 + 64, :], Alu.mult)
                        negw = post.tile([P, 512], f32, tag="negw")
                        nc.vector.scalar_tensor_tensor(
                            negw[:], in0=t2[:], scalar=lam_sb[:, 0:1],
                            in1=t1[:], op0=Alu.mult, op1=Alu.subtract)
                        sq = post.tile([P, 512], f32r, tag="sq")
                        nc.vector.tensor_tensor(sq[:], negw[:], negw[:], Alu.mult)
                        ms_ps = stp.tile([P, 512], f32, tag="st", name="msps")
                        nc.tensor.matmul(ms_ps[:], lhsT=hs_sb[:], rhs=sq[:],
                                         start=True, stop=True)
                        # per head: msb2 = ms/64 + EPS*(Z1*Z2)^2, broadcast
                        bcm = post.tile([P, 512], f32, tag="bcm")
                        ubm = post.tile([64, 512], f32, tag="ubm")
                        for hp in range(2):
                            r0 = hp * 64
                            z12 = small.tile([1, 512], f32, tag="z12")
                            nc.vector.tensor_tensor(
                                z12[:], zs[2 * hp][:], zs[2 * hp + 1][:],
                                Alu.mult)
                            msb = small.tile([1, 512], f32, tag="msb")
                            nc.vector.scalar_tensor_tensor(
                                msb[:], in0=z12[:], scalar=EPS, in1=z12[:],
                                op0=Alu.mult, op1=Alu.mult)
                            msb2 = small.tile([1, 512], f32, tag="msb2")
                            nc.vector.scalar_tensor_tensor(
                                msb2[:], in0=ms_ps[r0:r0 + 1, :],
                                scalar=1.0 / 64.0, in1=msb[:],
                                op0=Alu.mult, op1=Alu.add)
                            if hp == 0:
                                nc.gpsimd.partition_broadcast(bcm[0:64, :],
                                                              msb2[:])
                            else:
                                nc.gpsimd.partition_broadcast(ubm[:], msb2[:])
                                nc.vector.tensor_copy(bcm[64:128, :], ubm[:])
                        srt = post.tile([P, 512], f32, tag="srt")
                        nc.scalar.activation(srt[:], bcm[:], Act.Sqrt)
                        rstd = post.tile([P, 512], f32, tag="rstd")
                        nc.vector.reciprocal_approx_fast(rstd[:], srt[:])
                        nc.vector.scalar_tensor_tensor(
                            attnT[pt][:, q0:q0 + 512],
                            in0=negw[:], scalar=-(1.0 - LAMBDA_INIT),
                            in1=rstd[:], op0=Alu.mult, op1=Alu.mult)

                    # out-projection for this seq half (overlaps next qc)
                    for mo in range(8):
                        ps = stp.tile([P, 512], f32, tag="st", name="wops")
                        for kc in range(4):
                            nc.tensor.matmul(
                                ps[:],
                                lhsT=wo_sb[kc][:, mo * P:(mo + 1) * P],
                                rhs=attnT[kc][:, q0:q0 + 512],
                                start=(kc == 0), stop=(kc == 3))
                        po = posb.tile([P, 512], f32, tag="po")
                        nc.any.tensor_copy(po[:], ps[:])
                        nc.sync.dma_start(
                            po_dram[qc, mo * P:(mo + 1) * P, :], po[:])

            if debug:
                with tc.tile_pool(name="dbgp", bufs=2) as dbgp:
                    for i in range(4):
                        dq = dbgp.tile([P, S], f32, tag="dq", name="dq")
                        nc.vector.tensor_copy(dq[:], qT[i][:])
                        nc.sync.dma_start(dbg_q[i * P:(i + 1) * P, :], dq[:])
                        dk = dbgp.tile([P, S], f32, tag="dk", name="dk")
                        nc.vector.tensor_copy(dk[:], kT[i][:])
                        nc.sync.dma_start(dbg_k[i * P:(i + 1) * P, :], dk[:])
                        da = dbgp.tile([P, S], f32, tag="da", name="da")
                        nc.vector.tensor_copy(da[:], attnT[i][:])
                        nc.sync.dma_start(dbg_at[i * P:(i + 1) * P, :], da[:])
                    for i in range(8):
                        dv = dbgp.tile([P, 520], f32, tag="dv", name="dv")
                        nc.vector.tensor_copy(dv[:], va[i][:])
                        nc.sync.dma_start(dbg_va[i * P:(i + 1) * P, :], dv[:])

            _stk.close()

            nc.gpsimd.collective_compute(
                "ReduceScatter",
                mybir.AluOpType.add,
                replica_groups=[[0, 1], [2, 3], [4, 5], [6, 7]],
                ins=[po_dram.opt()],
                outs=[rs_dram.opt()],
            )

            # ---- phase 5: FFN + residual + final RMS on seq shard ----
            with (
                tc.tile_pool(name="aT", bufs=1) as atp,
                tc.tile_pool(name="astage", bufs=2) as astage,
                tc.tile_pool(name="h1", bufs=1) as h1p,
                tc.tile_pool(name="w1p", bufs=6) as w1p,
                tc.tile_pool(name="w2p", bufs=3) as w2p,
                tc.tile_pool(name="yT", bufs=1) as ytp,
                tc.tile_pool(name="fin", bufs=2) as finp,
                tc.tile_pool(name="sm2", bufs=1) as sm2,
            ):
                aTr = [atp.tile([P, 512], f32r, tag=f"ar{i}", name=f"ar{i}")
                       for i in range(8)]
                for i in range(8):
                    stg = astage.tile([P, 512], f32, tag="stg")
                    nc.sync.dma_start(stg[:], rs_dram[i * P:(i + 1) * P, :])
                    nc.vector.tensor_copy(aTr[i][:], stg[:])
                    if debug:
                        nc.sync.dma_start(dbg_rs[i * P:(i + 1) * P, :], stg[:])
                if debug:
                    nc.sync.dma_start(dbg_po[0:D, :], po_dram[0])
                    nc.sync.dma_start(dbg_po[D:2 * D, :], po_dram[1])

                h1 = [h1p.tile([P, 512], f32r, tag=f"h1_{i}", name=f"h1_{i}")
                      for i in range(32)]
                with tc.tile_pool(name="h1_ps", bufs=4, space="PSUM") as h1ps:
                    for mf in range(32):
                        wt = w1p.tile([P, 1024], f32r, tag="w1t", name="w1t")
                        nc.sync.dma_start(wt[:], w1s[mf, :, :])
                        ps = h1ps.tile([P, 512], f32, tag="h1ps", name="h1ps")
                        for kd in range(8):
                            nc.tensor.matmul(
                                ps[:], lhsT=wt[:, kd * P:(kd + 1) * P],
                                rhs=aTr[kd][:], start=(kd == 0), stop=(kd == 7))
                        nc.scalar.activation(h1[mf][:], ps[:], Act.Relu,
                                             bias=b1_sb[:, mf:mf + 1])

                # h2: 8 persistent PSUM accumulators, stream w2 tiles
                with tc.tile_pool(name="h2_ps", bufs=1, space="PSUM") as h2ps:
                    ps8 = [h2ps.tile([P, 512], f32, tag=f"h2_{mo}",
                                     name=f"h2_{mo}") for mo in range(8)]
                    for kf in range(32):
                        wt2 = w2p.tile([P, 1024], f32r, tag="w2t", name="w2t")
                        nc.sync.dma_start(wt2[:], w2T[kf * P:(kf + 1) * P, :])
                        for mo in range(8):
                            nc.tensor.matmul(
                                ps8[mo][:], lhsT=wt2[:, mo * P:(mo + 1) * P],
                                rhs=h1[kf][:], start=(kf == 0), stop=(kf == 31))
                    yt = [ytp.tile([P, 512], f32, tag=f"y{i}", name=f"y{i}")
                          for i in range(8)]
                    for mo in range(8):
                        nc.vector.scalar_tensor_tensor(
                            yt[mo][:], in0=ps8[mo][:],
                            scalar=b2_sb[:, mo:mo + 1], in1=aTr[mo][:],
                            op0=Alu.add, op1=Alu.add)

                # final RMS over D (partition dim across the 8 tiles)
                with tc.tile_pool(name="rms_ps", bufs=1, space="PSUM") as rmsps:
                    ms_ps = rmsps.tile([P, 512], f32, tag="rmsps", name="rmsps")
                    for mo in range(8):
                        sq = finp.tile([P, 512], f32r, tag="fsq", name="fsq")
                        nc.vector.tensor_tensor(sq[:], yt[mo][:], yt[mo][:],
                                                Alu.mult)
                        nc.tensor.matmul(ms_ps[0:1, :], lhsT=ones_r[:],
                                         rhs=sq[:], start=(mo == 0),
                                         stop=(mo == 7))
                    srt = sm2.tile([1, 512], f32, tag="fsrt")
                    nc.scalar.activation(srt[:], ms_ps[0:1, :], Act.Sqrt,
                                         scale=1.0 / 1024.0, bias=eps_sb[:])
                    rstd = sm2.tile([1, 512], f32, tag="frstd")
                    nc.vector.reciprocal_approx_fast(rstd[:], srt[:])
                    bcr = sm2.tile([P, 512], f32, tag="fbcr")
                    nc.gpsimd.partition_broadcast(bcr[:], rstd[:])
                    for mo in range(8):
                        ot = finp.tile([P, 512], f32, tag="fot", name="fot")
                        nc.vector.scalar_tensor_tensor(
                            ot[:], in0=yt[mo][:], scalar=rw_sb[:, mo:mo + 1],
                            in1=bcr[:], op0=Alu.mult, op1=Alu.mult)
                        nc.sync.dma_start(outT[mo * P:(mo + 1) * P, :], ot[:])

    nc.compile()
    return nc


def _host_prep(inputs):
    x = np.ascontiguousarray(np.asarray(inputs["x"], dtype=np.float32))
    Wq = np.asarray(inputs["Wq"], dtype=np.float32)
    Wk = np.asarray(inputs["Wk"], dtype=np.float32)
    Wv = np.asarray(inputs["Wv"], dtype=np.float32)
    Wo = np.asarray(inputs["Wo"], dtype=np.float32)
    W1 = np.asarray(inputs["W1"], dtype=np.float32)
    b1 = np.asarray(inputs["b1"], dtype=np.float32)
    W2 = np.asarray(inputs["W2"], dtype=np.float32)
    b2 = np.asarray(inputs["b2"], dtype=np.float32)
    rmsw = np.asarray(inputs["rms_weight"], dtype=np.float32)
    lam = float(np.exp(np.dot(np.asarray(inputs["lambda_q1"], np.float64),
                              np.asarray(inputs["lambda_k1"], np.float64)))
                - np.exp(np.dot(np.asarray(inputs["lambda_q2"], np.float64),
                                np.asarray(inputs["lambda_k2"], np.float64)))
                + LAMBDA_INIT)

    half = HD // 2
    cos16 = sin16 = None
    try:
        import jax.numpy as jnp
        freqs = 1.0 / (10000.0 ** (jnp.arange(half, dtype=jnp.float32) / half))
        ang = jnp.arange(S, dtype=jnp.float32)[:, None] * freqs[None, :]
        cos16 = np.asarray(jnp.cos(ang)).T.astype(np.float32)
        sin16 = np.asarray(jnp.sin(ang)).T.astype(np.float32)
    except Exception:
        pass
    if cos16 is None:
        freqs = (1.0 / (10000.0 ** (np.arange(half, dtype=np.float32)
                                    / np.float32(half)))).astype(np.float32)
        ang = (np.arange(S, dtype=np.float32)[:, None] * freqs[None, :])
        cos16 = np.cos(ang.astype(np.float32)).T.astype(np.float32)
        sin16 = np.sin(ang.astype(np.float32)).T.astype(np.float32)

    cosT = np.ascontiguousarray(np.tile(np.concatenate([cos16, cos16], 0), (4, 1)))
    sinS = np.ascontiguousarray(
        np.tile(np.concatenate([-sin16, sin16], 0), (4, 1))).astype(np.float32)
    perm32 = np.concatenate([np.arange(0, 32, 2), np.arange(1, 32, 2)])

    hsel = np.zeros((128, 128), np.float32)
    hsel[0:64, 0] = 1.0
    hsel[64:128, 64] = 1.0
    mdiag = (np.arange(128)[:, None] <= np.arange(128)[None, :]).astype(np.float32)

    b1c = np.ascontiguousarray(b1.reshape(32, 128).T)
    b2c = np.ascontiguousarray(b2.reshape(8, 128).T)
    rmswc = np.ascontiguousarray(rmsw.reshape(8, 128).T)
    lam128 = np.full((128, 1), lam, np.float32)
    # w1s[mf][p, kd*128+j] = W1.T[kd*128+p, mf*128+j]
    w1s = np.ascontiguousarray(
        W1.T.reshape(8, 128, 32, 128).transpose(2, 1, 0, 3).reshape(32, 128, 1024))
    w2T = np.ascontiguousarray(W2.T)

    in_maps = []
    for c in range(NCORES):
        b, g = c // 2, c % 2
        chans = np.arange(g * 512, (g + 1) * 512)
        permed = np.concatenate(
            [c0 * 32 + perm32 for c0 in range(g * 16, (g + 1) * 16)])
        in_maps.append({
            "xT": np.ascontiguousarray(x[b].T),
            "wqT": np.ascontiguousarray(Wq[permed, :].T),
            "wkT": np.ascontiguousarray(Wk[permed, :].T),
            "wvT": np.ascontiguousarray(Wv[chans, :].T),
            "woT": np.ascontiguousarray(Wo[:, chans].T),
            "w1s": w1s, "w2T": w2T,
            "b1c": b1c, "b2c": b2c, "rmswc": rmswc, "lam128": lam128,
            "cosT": cosT, "sinS": sinS,
            "hsel": hsel, "mdiag": mdiag,
        })
    return in_maps


def kernel(**inputs):
    global LAST_RESULT
    from concourse.bass_utils import run_bass_kernel_spmd

    key = "nc_dbg" if os.environ.get("KERNEL_DEBUG", "0") == "1" else "nc"
    if key not in _PROGRAM:
        _PROGRAM[key] = _build_program()
    nc = _PROGRAM[key]

    in_maps = _host_prep(inputs)
    trace = bool(int(os.environ.get("KERNEL_TRACE", "0")))
    res = run_bass_kernel_spmd(nc, in_maps, list(range(NCORES)), trace=trace)
    LAST_RESULT = res

    out = np.empty((B, S, D), np.float32)
    for c in range(NCORES):
        b, g = c // 2, c % 2
        out[b, g * 512:(g + 1) * 512, :] = res.results[c]["outT"].T
    return out


# revision 11
# speedup vs baseline: 1.1089x; 1.0485x over previous
"""DiffTransformer layer on 8 TRN2 NeuronCores.

Sharding: core c = (batch b=c//2, head-group g=c%2). Each core computes
q/k/v projections + differential attention for its 8 heads of its batch
(all in transposed [feature, seq] layout), a partial out-projection over
its 512 attention channels, then a pair ReduceScatter ([0,1],[2,3],...)
sums the two head-groups' partials and hands each core a 512-seq shard,
on which it runs the full FFN + residual + final RMSNorm.

Numerics: matmuls in float32r (TF32-like, ~11-bit mantissa, full rate on
the PE at moving-dim>=256) with fp32 PSUM accumulation. Softmax without
max-subtraction (scores bounded ~|4|), causal masking as an additive
-40 band folded into the score PSUM via an identity matmul, softmax
denominators via an appended ones-column on v (Z rides along in the PV
matmul), division deferred and folded into the subln RMS via scale
invariance (eps corrected by (Z1*Z2)^2). RoPE applied on PSUM eviction
via a stream-shuffle swap with host-permuted (evens-then-odds) q/k
weight rows.
"""
import os
import sys
import numpy as np

for _p in ("/opt/trn_rl_repo", "/root/.axon_site/_ro/trn_rl_repo"):
    if os.path.isdir(_p) and _p not in sys.path:
        sys.path.append(_p)

B, S, D, H, HD, FF = 4, 1024, 1024, 16, 32, 4096
NCORES = 8
LAMBDA_INIT = 0.8 - 0.6 * float(np.exp(-0.3 * 12))
EPS = 1e-5
SCALE = float(HD) ** -0.5

SWAP16 = [((i + 16) % 32) for i in range(32)]

LAST_RESULT = None  # BassKernelResults of the most recent run (for test.py)
_PROGRAM = {}


def _kts(qc):
    # (k-tile index, diag-band offset or None=full) for a 512-wide q chunk
    if qc == 0:
        return [(0, 0), (1, 128), (2, 256), (3, 384)]
    return [(0, None), (1, None), (2, None), (3, None),
            (4, 0), (5, 128), (6, 256), (7, 384)]


def _build_program():
    import concourse.bacc as bacc
    import concourse.mybir as mybir
    from concourse import tile
    from contextlib import ExitStack

    dt = mybir.dt
    f32, f32r = dt.float32, dt.float32r
    bf16 = dt.bfloat16
    Alu = mybir.AluOpType
    Act = mybir.ActivationFunctionType

    nc = bacc.Bacc("TRN2", target_bir_lowering=False, debug=False,
                   num_devices=NCORES)

    P = 128
    xT = nc.declare_dram_parameter("xT", [D, S], f32r, isOutput=False)
    wqT = nc.declare_dram_parameter("wqT", [D, 512], f32r, isOutput=False)
    wkT = nc.declare_dram_parameter("wkT", [D, 512], f32r, isOutput=False)
    wvT = nc.declare_dram_parameter("wvT", [D, 512], f32r, isOutput=False)
    woT = nc.declare_dram_parameter("woT", [512, D], f32r, isOutput=False)
    w1s = nc.declare_dram_parameter("w1s", [32, P, 1024], f32r, isOutput=False)
    w2T = nc.declare_dram_parameter("w2T", [FF, D], f32r, isOutput=False)
    b1c = nc.declare_dram_parameter("b1c", [P, 32], f32, isOutput=False)
    b2c = nc.declare_dram_parameter("b2c", [P, 8], f32, isOutput=False)
    rmswc = nc.declare_dram_parameter("rmswc", [P, 8], f32, isOutput=False)
    lam128 = nc.declare_dram_parameter("lam128", [P, 1], f32, isOutput=False)
    cosT = nc.declare_dram_parameter("cosT", [P, S], f32, isOutput=False)
    sinS = nc.declare_dram_parameter("sinS", [P, S], f32, isOutput=False)
    hsel = nc.declare_dram_parameter("hsel", [P, P], f32r, isOutput=False)
    mdiag = nc.declare_dram_parameter("mdiag", [P, P], f32, isOutput=False)
    outT = nc.declare_dram_parameter("outT", [D, 512], f32, isOutput=True)
    debug = bool(int(os.environ.get("KERNEL_DEBUG", "0")))
    if debug:
        dbg_q = nc.declare_dram_parameter("dbg_q", [512, S], f32, isOutput=True)
        dbg_k = nc.declare_dram_parameter("dbg_k", [512, S], f32, isOutput=True)
        dbg_va = nc.declare_dram_parameter("dbg_va", [8 * P, 520], f32, isOutput=True)
        dbg_at = nc.declare_dram_parameter("dbg_at", [512, S], f32, isOutput=True)
        dbg_rs = nc.declare_dram_parameter("dbg_rs", [D, 512], f32, isOutput=True)
        dbg_po = nc.declare_dram_parameter("dbg_po", [2 * D, 512], f32, isOutput=True)

    with tile.TileContext(nc) as tc:
        with (
            tc.tile_pool(name="consts", bufs=1) as consts,
            tc.tile_pool(name="dram", bufs=1, space="DRAM") as dram,
        ):
            # ---- constants -------------------------------------------
            cos_sb = consts.tile([P, S], f32, tag="cos")
            sin_sb = consts.tile([P, S], f32, tag="sin")
            hs_sb = consts.tile([P, P], f32r, tag="hs")
            md_sb = consts.tile([P, P], f32, tag="md")
            lam_sb = consts.tile([P, 1], f32, tag="lam")
            b1_sb = consts.tile([P, 32], f32, tag="b1")
            b2_sb = consts.tile([P, 8], f32, tag="b2")
            rw_sb = consts.tile([P, 8], f32, tag="rw")
            wo_sb = [consts.tile([P, D], f32r, tag=f"wo{i}", name=f"wo{i}")
                     for i in range(4)]
            nc.sync.dma_start(cos_sb[:], cosT[:])
            nc.sync.dma_start(sin_sb[:], sinS[:])
            nc.sync.dma_start(hs_sb[:], hsel[:])
            nc.sync.dma_start(md_sb[:], mdiag[:])
            nc.sync.dma_start(lam_sb[:], lam128[:])
            nc.sync.dma_start(b1_sb[:], b1c[:])
            nc.sync.dma_start(b2_sb[:], b2c[:])
            nc.sync.dma_start(rw_sb[:], rmswc[:])
            for i in range(4):
                nc.sync.dma_start(wo_sb[i][:], woT[i * P:(i + 1) * P, :])
            ones_f32 = consts.tile([P, 8], f32, tag="onesf")
            nc.vector.memset(ones_f32[:], 1.0)
            eps_sb = consts.tile([1, 1], f32, tag="eps")
            nc.vector.memset(eps_sb[:], EPS)
            ones_r = consts.tile([P, 1], f32r, tag="onesr")
            nc.vector.tensor_copy(ones_r[:], ones_f32[:, 0:1])

            _stk = ExitStack()
            qkpool = _stk.enter_context(tc.tile_pool(name="qk", bufs=1))
            vapool = _stk.enter_context(tc.tile_pool(name="vaug", bufs=1))
            attnpool = _stk.enter_context(tc.tile_pool(name="attn", bufs=1))

            qT = [qkpool.tile([P, S], f32r, tag=f"qT{i}", name=f"qT{i}")
                  for i in range(4)]
            kT = [qkpool.tile([P, S], f32r, tag=f"kT{i}", name=f"kT{i}")
                  for i in range(4)]
            va = [vapool.tile([P, 8 * 65], f32r, tag=f"va{i}", name=f"va{i}")
                  for i in range(8)]
            attnT = [attnpool.tile([P, S], f32r, tag=f"at{i}", name=f"at{i}")
                     for i in range(4)]

            # [D-half][pair-chunk][512 D rows][512 seq] so each RS half is
            # a contiguous block
            po_dram = dram.tile([2, 2, 512, 512], bf16)
            rs_dram = dram.tile([2, 512, 512], bf16)

            # ---- phase 1+2: load xT / weights, project v then q,k ----
            with (
                tc.tile_pool(name="xw", bufs=1) as xw,
                tc.tile_pool(name="proj_ps", bufs=6, space="PSUM") as pps,
                tc.tile_pool(name="rtmp", bufs=4) as rtmp,
            ):
                xt = [xw.tile([P, S], f32r, tag=f"x{i}", name=f"x{i}")
                      for i in range(8)]
                wq_sb = [xw.tile([P, 512], f32r, tag=f"wq{i}", name=f"wqs{i}")
                         for i in range(8)]
                wk_sb = [xw.tile([P, 512], f32r, tag=f"wk{i}", name=f"wks{i}")
                         for i in range(8)]
                wv_sb = [xw.tile([P, 512], f32r, tag=f"wv{i}", name=f"wvs{i}")
                         for i in range(8)]
                for i in range(8):
                    nc.sync.dma_start(xt[i][:], xT[i * P:(i + 1) * P, :])
                    nc.sync.dma_start(wv_sb[i][:], wvT[i * P:(i + 1) * P, :])
                    nc.sync.dma_start(wq_sb[i][:], wqT[i * P:(i + 1) * P, :])
                    nc.sync.dma_start(wk_sb[i][:], wkT[i * P:(i + 1) * P, :])

                def project_v(st):
                    ps = pps.tile([P, 512], f32, tag="ps", name="ps")
                    for kd in range(8):
                        nc.tensor.matmul(
                            ps[:],
                            lhsT=xt[kd][:, st * P:(st + 1) * P],
                            rhs=wv_sb[kd][:],
                            start=(kd == 0), stop=(kd == 7))
                    va3 = va[st][:].rearrange("p (h e) -> p h e", h=8, e=65)
                    nc.any.tensor_copy(
                        va3[:, :, 0:64],
                        ps[:].rearrange("p (h e) -> p h e", h=8, e=64))
                    nc.any.tensor_copy(
                        va3[:, :, 64:65],
                        ones_f32[:].rearrange("p (h o) -> p h o", o=1))

                def project_qk(mt, nch):
                    n0 = nch * 512
                    for wsb, dstT in ((wq_sb, qT), (wk_sb, kT)):
                        ps = pps.tile([P, 512], f32, tag="ps", name="ps")
                        for kd in range(8):
                            nc.tensor.matmul(
                                ps[:],
                                lhsT=wsb[kd][:, mt * P:(mt + 1) * P],
                                rhs=xt[kd][:, n0:n0 + 512],
                                start=(kd == 0), stop=(kd == 7))
                        dst = dstT[mt][:, n0:n0 + 512]
                        nc.vector.tensor_tensor(
                            dst, ps[:], cos_sb[:, n0:n0 + 512], Alu.mult)
                        tmp = rtmp.tile([P, 512], f32, tag="rt", name="rt")
                        nc.vector.stream_shuffle(tmp[:], ps[:], SWAP16)
                        nc.vector.tensor_tensor(
                            tmp[:], tmp[:], sin_sb[:, n0:n0 + 512], Alu.mult)
                        nc.vector.tensor_tensor(dst, dst, tmp[:], Alu.add)

                # qc0 needs q/k cols 0:512 (nch0) + va; emit those first
                for mt in range(4):
                    project_qk(mt, 0)
                    project_v(2 * mt)
                    project_v(2 * mt + 1)
                for mt in range(4):
                    project_qk(mt, 1)

            # ---- phase 3: differential attention + out-projection ----
            with (
                tc.tile_pool(name="pv_ps", bufs=4, space="PSUM") as pvp,
                tc.tile_pool(name="st_ps", bufs=4, space="PSUM") as stp,
                tc.tile_pool(name="epool", bufs=6) as epool,
                tc.tile_pool(name="post", bufs=2) as post,
                tc.tile_pool(name="small", bufs=3) as small,
                tc.tile_pool(name="po_sb", bufs=4) as posb,
            ):
                for qc in range(2):
                    q0 = qc * 512
                    kts = _kts(qc)
                    last_kt = kts[-1][0]
                    for pt in range(4):
                        pvs = [pvp.tile([P, 512], f32, tag="pv", name="pv")
                               for _ in range(4)]
                        for kt, off in kts:
                            j0 = 0 if off is None else off
                            sts = []
                            for gq in range(4):
                                st_ps = stp.tile([P, 512], f32, tag="st",
                                                 name="st")
                                nc.tensor.matmul(
                                    st_ps[:, j0:],
                                    lhsT=kT[pt][gq * 32:(gq + 1) * 32,
                                                kt * P:(kt + 1) * P],
                                    rhs=qT[pt][gq * 32:(gq + 1) * 32,
                                               q0 + j0:q0 + 512],
                                    start=True, stop=True,
                                    tile_position=(gq * 32, 0))
                                sts.append(st_ps)
                            for gq in range(4):
                                e = epool.tile([P, 512], f32r, tag="e",
                                               name="e")
                                nc.scalar.activation(
                                    e[:, j0:], sts[gq][:, j0:], Act.Exp,
                                    scale=SCALE)
                                if off is not None:
                                    nc.vector.tensor_tensor(
                                        e[:, j0:j0 + P], e[:, j0:j0 + P],
                                        md_sb[:], Alu.mult)
                                h_loc = (pt * P + gq * 32) // 64
                                nc.tensor.matmul(
                                    pvs[gq][0:65, j0:],
                                    lhsT=va[kt][:, h_loc * 65:(h_loc + 1) * 65],
                                    rhs=e[:, j0:],
                                    start=(kt == 0), stop=(kt == last_kt))

                        # free the PV banks ASAP: stage A and Z to SBUF
                        a1 = post.tile([P, 512], f32, tag="a1")
                        a2 = post.tile([P, 512], f32, tag="a2")
                        zs = [small.tile([1, 512], f32, tag=f"z{i}",
                                         name=f"z{i}") for i in range(4)]
                        for hp in range(2):
                            nc.vector.tensor_copy(a1[hp * 64:(hp + 1) * 64, :],
                                                  pvs[2 * hp][0:64, :])
                            nc.vector.tensor_copy(a2[hp * 64:(hp + 1) * 64, :],
                                                  pvs[2 * hp + 1][0:64, :])
                            nc.vector.tensor_copy(zs[2 * hp][:],
                                                  pvs[2 * hp][64:65, :])
                            nc.vector.tensor_copy(zs[2 * hp + 1][:],
                                                  pvs[2 * hp + 1][64:65, :])
                        bc1 = post.tile([P, 512], f32, tag="bc1")
                        bc2 = post.tile([P, 512], f32, tag="bc2")
                        ubc = post.tile([64, 512], f32, tag="ubc")
                        ubc2 = post.tile([64, 512], f32, tag="ubc2")
                        nc.gpsimd.partition_broadcast(bc1[0:64, :], zs[0][:])
                        nc.gpsimd.partition_broadcast(ubc[:], zs[2][:])
                        nc.vector.tensor_copy(bc1[64:128, :], ubc[:])
                        nc.gpsimd.partition_broadcast(bc2[0:64, :], zs[1][:])
                        nc.gpsimd.partition_broadcast(ubc2[:], zs[3][:])
                        nc.vector.tensor_copy(bc2[64:128, :], ubc2[:])
                        t1 = post.tile([P, 512], f32, tag="t1")
                        t2 = post.tile([P, 512], f32, tag="t2")
                        nc.vector.tensor_tensor(t1[:], a1[:], bc2[:], Alu.mult)
                        nc.vector.tensor_tensor(t2[:], a2[:], bc1[:], Alu.mult)
                        negw = post.tile([P, 512], f32, tag="negw")
                        nc.vector.scalar_tensor_tensor(
                            negw[:], in0=t2[:], scalar=lam_sb[:, 0:1],
                            in1=t1[:], op0=Alu.mult, op1=Alu.subtract)
                        sq = post.tile([P, 512], f32r, tag="sq")
                        nc.vector.tensor_tensor(sq[:], negw[:], negw[:], Alu.mult)
                        ms_ps = stp.tile([P, 512], f32, tag="st", name="msps")
                        nc.tensor.matmul(ms_ps[:], lhsT=hs_sb[:], rhs=sq[:],
                                         start=True, stop=True)
                        # per head: msb2 = ms/64 + EPS*(Z1*Z2)^2, broadcast
                        bcm = post.tile([P, 512], f32, tag="bcm")
                        ubm = post.tile([64, 512], f32, tag="ubm")
                        for hp in range(2):
                            r0 = hp * 64
                            z12 = small.tile([1, 512], f32, tag="z12")
                            nc.vector.tensor_tensor(
                                z12[:], zs[2 * hp][:], zs[2 * hp + 1][:],
                                Alu.mult)
                            msb = small.tile([1, 512], f32, tag="msb")
                            nc.vector.scalar_tensor_tensor(
                                msb[:], in0=z12[:], scalar=EPS, in1=z12[:],
                                op0=Alu.mult, op1=Alu.mult)
                            msb2 = small.tile([1, 512], f32, tag="msb2")
                            nc.vector.scalar_tensor_tensor(
                                msb2[:], in0=ms_ps[r0:r0 + 1, :],
                                scalar=1.0 / 64.0, in1=msb[:],
                                op0=Alu.mult, op1=Alu.add)
                            if hp == 0:
                                nc.gpsimd.partition_broadcast(bcm[0:64, :],
                                                              msb2[:])
                            else:
                                nc.gpsimd.partition_broadcast(ubm[:], msb2[:])
                                nc.vector.tensor_copy(bcm[64:128, :], ubm[:])
                        srt = post.tile([P, 512], f32, tag="srt")
                        nc.scalar.activation(srt[:], bcm[:], Act.Sqrt)
                        rstd = post.tile([P, 512], f32, tag="rstd")
                        nc.vector.reciprocal_approx_fast(rstd[:], srt[:])
                        nc.vector.scalar_tensor_tensor(
                            attnT[pt][:, q0:q0 + 512],
                            in0=negw[:], scalar=-(1.0 - LAMBDA_INIT),
                            in1=rstd[:], op0=Alu.mult, op1=Alu.mult)

                    # out-projection for this seq half (overlaps next qc)
                    for mo in range(8):
                        ps = stp.tile([P, 512], f32, tag="st", name="wops")
                        for kc in range(4):
                            nc.tensor.matmul(
                                ps[:],
                                lhsT=wo_sb[kc][:, mo * P:(mo + 1) * P],
                                rhs=attnT[kc][:, q0:q0 + 512],
                                start=(kc == 0), stop=(kc == 3))
                        po = posb.tile([P, 512], bf16, tag="po")
                        nc.any.tensor_copy(po[:], ps[:])
                        nc.sync.dma_start(
                            po_dram[mo // 4, qc,
                                    (mo % 4) * P:(mo % 4 + 1) * P, :], po[:])

            if debug:
                with tc.tile_pool(name="dbgp", bufs=2) as dbgp:
                    for i in range(4):
                        dq = dbgp.tile([P, S], f32, tag="dq", name="dq")
                        nc.vector.tensor_copy(dq[:], qT[i][:])
                        nc.sync.dma_start(dbg_q[i * P:(i + 1) * P, :], dq[:])
                        dk = dbgp.tile([P, S], f32, tag="dk", name="dk")
                        nc.vector.tensor_copy(dk[:], kT[i][:])
                        nc.sync.dma_start(dbg_k[i * P:(i + 1) * P, :], dk[:])
                        da = dbgp.tile([P, S], f32, tag="da", name="da")
                        nc.vector.tensor_copy(da[:], attnT[i][:])
                        nc.sync.dma_start(dbg_at[i * P:(i + 1) * P, :], da[:])
                    for i in range(8):
                        dv = dbgp.tile([P, 520], f32, tag="dv", name="dv")
                        nc.vector.tensor_copy(dv[:], va[i][:])
                        nc.sync.dma_start(dbg_va[i * P:(i + 1) * P, :], dv[:])

            _stk.close()

            for dh in range(2):
                nc.gpsimd.collective_compute(
                    "ReduceScatter",
                    mybir.AluOpType.add,
                    replica_groups=[[0, 1], [2, 3], [4, 5], [6, 7]],
                    ins=[po_dram[dh].opt()],
                    outs=[rs_dram[dh].opt()],
                )

            # ---- phase 5: FFN + residual + final RMS on seq shard ----
            with (
                tc.tile_pool(name="aT", bufs=1) as atp,
                tc.tile_pool(name="astage", bufs=2) as astage,
                tc.tile_pool(name="h1", bufs=1) as h1p,
                tc.tile_pool(name="w1p", bufs=6) as w1p,
                tc.tile_pool(name="w2p", bufs=3) as w2p,
                tc.tile_pool(name="yT", bufs=1) as ytp,
                tc.tile_pool(name="fin", bufs=2) as finp,
                tc.tile_pool(name="sm2", bufs=1) as sm2,
            ):
                aTr = [atp.tile([P, 512], f32r, tag=f"ar{i}", name=f"ar{i}")
                       for i in range(8)]
                for i in range(8):
                    stg = astage.tile([P, 512], bf16, tag="stg")
                    nc.sync.dma_start(
                        stg[:], rs_dram[i // 4, (i % 4) * P:(i % 4 + 1) * P, :])
                    nc.vector.tensor_copy(aTr[i][:], stg[:])
                    if debug:
                        nc.gpsimd.dma_start(dbg_rs[i * P:(i + 1) * P, :], stg[:])
                if debug:
                    for dh in range(2):
                        for r in range(2):
                            nc.gpsimd.dma_start(
                                dbg_po[r * D + dh * 512:r * D + (dh + 1) * 512, :],
                                po_dram[dh, r])

                h1 = [h1p.tile([P, 512], f32r, tag=f"h1_{i}", name=f"h1_{i}")
                      for i in range(32)]
                with tc.tile_pool(name="h1_ps", bufs=4, space="PSUM") as h1ps:
                    for mf in range(32):
                        wt = w1p.tile([P, 1024], f32r, tag="w1t", name="w1t")
                        nc.sync.dma_start(wt[:], w1s[mf, :, :])
                        ps = h1ps.tile([P, 512], f32, tag="h1ps", name="h1ps")
                        for kd in range(8):
                            nc.tensor.matmul(
                                ps[:], lhsT=wt[:, kd * P:(kd + 1) * P],
                                rhs=aTr[kd][:], start=(kd == 0), stop=(kd == 7))
                        nc.scalar.activation(h1[mf][:], ps[:], Act.Relu,
                                             bias=b1_sb[:, mf:mf + 1])

                # h2: 8 persistent PSUM accumulators, stream w2 tiles
                with tc.tile_pool(name="h2_ps", bufs=1, space="PSUM") as h2ps:
                    ps8 = [h2ps.tile([P, 512], f32, tag=f"h2_{mo}",
                                     name=f"h2_{mo}") for mo in range(8)]
                    for kf in range(32):
                        wt2 = w2p.tile([P, 1024], f32r, tag="w2t", name="w2t")
                        nc.sync.dma_start(wt2[:], w2T[kf * P:(kf + 1) * P, :])
                        for mo in range(8):
                            nc.tensor.matmul(
                                ps8[mo][:], lhsT=wt2[:, mo * P:(mo + 1) * P],
                                rhs=h1[kf][:], start=(kf == 0), stop=(kf == 31))
                    yt = [ytp.tile([P, 512], f32, tag=f"y{i}", name=f"y{i}")
                          for i in range(8)]
                    for mo in range(8):
                        nc.vector.scalar_tensor_tensor(
                            yt[mo][:], in0=ps8[mo][:],
                            scalar=b2_sb[:, mo:mo + 1], in1=aTr[mo][:],
                            op0=Alu.add, op1=Alu.add)

                # final RMS over D (partition dim across the 8 tiles)
                with tc.tile_pool(name="rms_ps", bufs=1, space="PSUM") as rmsps:
                    ms_ps = rmsps.tile([P, 512], f32, tag="rmsps", name="rmsps")
                    for mo in range(8):
                        sq = finp.tile([P, 512], f32r, tag="fsq", name="fsq")
                        nc.vector.tensor_tensor(sq[:], yt[mo][:], yt[mo][:],
                                                Alu.mult)
                        nc.tensor.matmul(ms_ps[0:1, :], lhsT=ones_r[:],
                                         rhs=sq[:], start=(mo == 0),
                                         stop=(mo == 7))
                    srt = sm2.tile([1, 512], f32, tag="fsrt")
                    nc.scalar.activation(srt[:], ms_ps[0:1, :], Act.Sqrt,
                                         scale=1.0 / 1024.0, bias=eps_sb[:])
                    rstd = sm2.tile([1, 512], f32, tag="frstd")
                    nc.vector.reciprocal_approx_fast(rstd[:], srt[:])
                    bcr = sm2.tile([P, 512], f32, tag="fbcr")
                    nc.gpsimd.partition_broadcast(bcr[:], rstd[:])
                    for mo in range(8):
                        ot = finp.tile([P, 512], f32, tag="fot", name="fot")
                        nc.vector.scalar_tensor_tensor(
                            ot[:], in0=yt[mo][:], scalar=rw_sb[:, mo:mo + 1],
                            in1=bcr[:], op0=Alu.mult, op1=Alu.mult)
                        nc.sync.dma_start(outT[mo * P:(mo + 1) * P, :], ot[:])

    nc.compile()
    return nc


def _host_prep(inputs):
    x = np.ascontiguousarray(np.asarray(inputs["x"], dtype=np.float32))
    Wq = np.asarray(inputs["Wq"], dtype=np.float32)
    Wk = np.asarray(inputs["Wk"], dtype=np.float32)
    Wv = np.asarray(inputs["Wv"], dtype=np.float32)
    Wo = np.asarray(inputs["Wo"], dtype=np.float32)
    W1 = np.asarray(inputs["W1"], dtype=np.float32)
    b1 = np.asarray(inputs["b1"], dtype=np.float32)
    W2 = np.asarray(inputs["W2"], dtype=np.float32)
    b2 = np.asarray(inputs["b2"], dtype=np.float32)
    rmsw = np.asarray(inputs["rms_weight"], dtype=np.float32)
    lam = float(np.exp(np.dot(np.asarray(inputs["lambda_q1"], np.float64),
                              np.asarray(inputs["lambda_k1"], np.float64)))
                - np.exp(np.dot(np.asarray(inputs["lambda_q2"], np.float64),
                                np.asarray(inputs["lambda_k2"], np.float64)))
                + LAMBDA_INIT)

    half = HD // 2
    cos16 = sin16 = None
    try:
        import jax.numpy as jnp
        freqs = 1.0 / (10000.0 ** (jnp.arange(half, dtype=jnp.float32) / half))
        ang = jnp.arange(S, dtype=jnp.float32)[:, None] * freqs[None, :]
        cos16 = np.asarray(jnp.cos(ang)).T.astype(np.float32)
        sin16 = np.asarray(jnp.sin(ang)).T.astype(np.float32)
    except Exception:
        pass
    if cos16 is None:
        freqs = (1.0 / (10000.0 ** (np.arange(half, dtype=np.float32)
                                    / np.float32(half)))).astype(np.float32)
        ang = (np.arange(S, dtype=np.float32)[:, None] * freqs[None, :])
        cos16 = np.cos(ang.astype(np.float32)).T.astype(np.float32)
        sin16 = np.sin(ang.astype(np.float32)).T.astype(np.float32)

    cosT = np.ascontiguousarray(np.tile(np.concatenate([cos16, cos16], 0), (4, 1)))
    sinS = np.ascontiguousarray(
        np.tile(np.concatenate([-sin16, sin16], 0), (4, 1))).astype(np.float32)
    perm32 = np.concatenate([np.arange(0, 32, 2), np.arange(1, 32, 2)])

    hsel = np.zeros((128, 128), np.float32)
    hsel[0:64, 0] = 1.0
    hsel[64:128, 64] = 1.0
    mdiag = (np.arange(128)[:, None] <= np.arange(128)[None, :]).astype(np.float32)

    b1c = np.ascontiguousarray(b1.reshape(32, 128).T)
    b2c = np.ascontiguousarray(b2.reshape(8, 128).T)
    rmswc = np.ascontiguousarray(rmsw.reshape(8, 128).T)
    lam128 = np.full((128, 1), lam, np.float32)
    # w1s[mf][p, kd*128+j] = W1.T[kd*128+p, mf*128+j]
    w1s = np.ascontiguousarray(
        W1.T.reshape(8, 128, 32, 128).transpose(2, 1, 0, 3).reshape(32, 128, 1024))
    w2T = np.ascontiguousarray(W2.T)

    in_maps = []
    for c in range(NCORES):
        b, g = c // 2, c % 2
        chans = np.arange(g * 512, (g + 1) * 512)
        permed = np.concatenate(
            [c0 * 32 + perm32 for c0 in range(g * 16, (g + 1) * 16)])
        in_maps.append({
            "xT": np.ascontiguousarray(x[b].T),
            "wqT": np.ascontiguousarray(Wq[permed, :].T),
            "wkT": np.ascontiguousarray(Wk[permed, :].T),
            "wvT": np.ascontiguousarray(Wv[chans, :].T),
            "woT": np.ascontiguousarray(Wo[:, chans].T),
            "w1s": w1s, "w2T": w2T,
            "b1c": b1c, "b2c": b2c, "rmswc": rmswc, "lam128": lam128,
            "cosT": cosT, "sinS": sinS,
            "hsel": hsel, "mdiag": mdiag,
        })
    return in_maps


def kernel(**inputs):
    global LAST_RESULT
    from concourse.bass_utils import run_bass_kernel_spmd

    key = "nc_dbg" if os.environ.get("KERNEL_DEBUG", "0") == "1" else "nc"
    if key not in _PROGRAM:
        _PROGRAM[key] = _build_program()
    nc = _PROGRAM[key]

    in_maps = _host_prep(inputs)
    trace = bool(int(os.environ.get("KERNEL_TRACE", "0")))
    res = run_bass_kernel_spmd(nc, in_maps, list(range(NCORES)), trace=trace)
    LAST_RESULT = res

    out = np.empty((B, S, D), np.float32)
    for c in range(NCORES):
        b, g = c // 2, c % 2
        out[b, g * 512:(g + 1) * 512, :] = res.results[c]["outT"].T
    return out


# revision 13
# speedup vs baseline: 1.1736x; 1.0583x over previous
"""DiffTransformer layer on 8 TRN2 NeuronCores.

Sharding: core c = (batch b=c//2, head-group g=c%2). Each core computes
q/k/v projections + differential attention for its 8 heads of its batch
(all in transposed [feature, seq] layout), a partial out-projection over
its 512 attention channels, then a pair ReduceScatter ([0,1],[2,3],...)
sums the two head-groups' partials and hands each core a 512-seq shard,
on which it runs the full FFN + residual + final RMSNorm.

Numerics: matmuls in float32r (TF32-like, ~11-bit mantissa, full rate on
the PE at moving-dim>=256) with fp32 PSUM accumulation. Softmax without
max-subtraction (scores bounded ~|4|), causal masking as an additive
-40 band folded into the score PSUM via an identity matmul, softmax
denominators via an appended ones-column on v (Z rides along in the PV
matmul), division deferred and folded into the subln RMS via scale
invariance (eps corrected by (Z1*Z2)^2). RoPE applied on PSUM eviction
via a stream-shuffle swap with host-permuted (evens-then-odds) q/k
weight rows.
"""
import os
import sys
import numpy as np

for _p in ("/opt/trn_rl_repo", "/root/.axon_site/_ro/trn_rl_repo"):
    if os.path.isdir(_p) and _p not in sys.path:
        sys.path.append(_p)

B, S, D, H, HD, FF = 4, 1024, 1024, 16, 32, 4096
NCORES = 8
LAMBDA_INIT = 0.8 - 0.6 * float(np.exp(-0.3 * 12))
EPS = 1e-5
SCALE = float(HD) ** -0.5

SWAP16 = [((i + 16) % 32) for i in range(32)]

LAST_RESULT = None  # BassKernelResults of the most recent run (for test.py)
_PROGRAM = {}


def _kts(qc):
    # (k-tile index, diag-band offset or None=full) for a 512-wide q chunk
    if qc == 0:
        return [(0, 0), (1, 128), (2, 256), (3, 384)]
    return [(0, None), (1, None), (2, None), (3, None),
            (4, 0), (5, 128), (6, 256), (7, 384)]


def _build_program():
    import concourse.bacc as bacc
    import concourse.mybir as mybir
    from concourse import tile
    from contextlib import ExitStack

    dt = mybir.dt
    f32, f32r = dt.float32, dt.float32r
    bf16 = dt.bfloat16
    Alu = mybir.AluOpType
    Act = mybir.ActivationFunctionType

    nc = bacc.Bacc("TRN2", target_bir_lowering=False, debug=False,
                   num_devices=NCORES)

    P = 128
    xT = nc.declare_dram_parameter("xT", [D, S], f32r, isOutput=False)
    wqT = nc.declare_dram_parameter("wqT", [D, 512], f32r, isOutput=False)
    wkT = nc.declare_dram_parameter("wkT", [D, 512], f32r, isOutput=False)
    wvT = nc.declare_dram_parameter("wvT", [D, 512], f32r, isOutput=False)
    woT = nc.declare_dram_parameter("woT", [512, D], f32r, isOutput=False)
    w1s = nc.declare_dram_parameter("w1s", [32, P, 1024], f32r, isOutput=False)
    w2T = nc.declare_dram_parameter("w2T", [FF, D], f32r, isOutput=False)
    b1c = nc.declare_dram_parameter("b1c", [P, 32], f32, isOutput=False)
    b2c = nc.declare_dram_parameter("b2c", [P, 8], f32, isOutput=False)
    rmswc = nc.declare_dram_parameter("rmswc", [P, 8], f32, isOutput=False)
    lam128 = nc.declare_dram_parameter("lam128", [P, 1], f32, isOutput=False)
    cosT = nc.declare_dram_parameter("cosT", [P, S], f32, isOutput=False)
    sinS = nc.declare_dram_parameter("sinS", [P, S], f32, isOutput=False)
    hsel = nc.declare_dram_parameter("hsel", [P, P], f32r, isOutput=False)
    mdiag = nc.declare_dram_parameter("mdiag", [P, P], f32, isOutput=False)
    outT = nc.declare_dram_parameter("outT", [D, 512], f32, isOutput=True)
    debug = bool(int(os.environ.get("KERNEL_DEBUG", "0")))
    if debug:
        dbg_q = nc.declare_dram_parameter("dbg_q", [512, S], f32, isOutput=True)
        dbg_k = nc.declare_dram_parameter("dbg_k", [512, S], f32, isOutput=True)
        dbg_va = nc.declare_dram_parameter("dbg_va", [8 * P, 520], f32, isOutput=True)
        dbg_at = nc.declare_dram_parameter("dbg_at", [512, S], f32, isOutput=True)
        dbg_rs = nc.declare_dram_parameter("dbg_rs", [D, 512], f32, isOutput=True)
        dbg_po = nc.declare_dram_parameter("dbg_po", [2 * D, 512], f32, isOutput=True)

    with tile.TileContext(nc) as tc:
        with (
            tc.tile_pool(name="consts", bufs=1) as consts,
            tc.tile_pool(name="dram", bufs=1, space="DRAM") as dram,
        ):
            # ---- constants -------------------------------------------
            hs_sb = consts.tile([P, P], f32r, tag="hs")
            md_sb = consts.tile([P, P], f32, tag="md")
            lam_sb = consts.tile([P, 1], f32, tag="lam")
            b1_sb = consts.tile([P, 32], f32, tag="b1")
            b2_sb = consts.tile([P, 8], f32, tag="b2")
            rw_sb = consts.tile([P, 8], f32, tag="rw")
            wo_sb = [consts.tile([P, D], f32r, tag=f"wo{i}", name=f"wo{i}")
                     for i in range(4)]
            nc.sync.dma_start(hs_sb[:], hsel[:])
            nc.sync.dma_start(md_sb[:], mdiag[:])
            nc.sync.dma_start(lam_sb[:], lam128[:])
            nc.sync.dma_start(b1_sb[:], b1c[:])
            nc.sync.dma_start(b2_sb[:], b2c[:])
            nc.sync.dma_start(rw_sb[:], rmswc[:])
            for i in range(4):
                nc.sync.dma_start(wo_sb[i][:], woT[i * P:(i + 1) * P, :])
            ones_f32 = consts.tile([P, 8], f32, tag="onesf")
            nc.vector.memset(ones_f32[:], 1.0)
            eps_sb = consts.tile([1, 1], f32, tag="eps")
            nc.vector.memset(eps_sb[:], EPS)
            ones_r = consts.tile([P, 1], f32r, tag="onesr")
            nc.vector.tensor_copy(ones_r[:], ones_f32[:, 0:1])

            _stk = ExitStack()
            qkpool = _stk.enter_context(tc.tile_pool(name="qk", bufs=1))
            vapool = _stk.enter_context(tc.tile_pool(name="vaug", bufs=1))
            attnpool = _stk.enter_context(tc.tile_pool(name="attn", bufs=1))

            qT = [qkpool.tile([P, S], f32r, tag=f"qT{i}", name=f"qT{i}")
                  for i in range(4)]
            kT = [qkpool.tile([P, S], f32r, tag=f"kT{i}", name=f"kT{i}")
                  for i in range(4)]
            va = [vapool.tile([P, 8 * 65], f32r, tag=f"va{i}", name=f"va{i}")
                  for i in range(8)]
            attnT = [attnpool.tile([P, S], f32r, tag=f"at{i}", name=f"at{i}")
                     for i in range(4)]

            # [D-half][pair-chunk][512 D rows][512 seq] so each RS half is
            # a contiguous block
            po_dram = dram.tile([2, 2, 512, 512], bf16)
            rs_dram = dram.tile([2, 512, 512], bf16)

            # ---- phase 1+2: load xT / weights, project v then q,k ----
            with (
                tc.tile_pool(name="xw", bufs=1) as xw,
                tc.tile_pool(name="proj_ps", bufs=6, space="PSUM") as pps,
                tc.tile_pool(name="rtmp", bufs=4) as rtmp,
            ):
                xt = [xw.tile([P, S], f32r, tag=f"x{i}", name=f"x{i}")
                      for i in range(8)]
                cos_sb = xw.tile([P, S], f32, tag="cos")
                sin_sb = xw.tile([P, S], f32, tag="sin")
                nc.sync.dma_start(cos_sb[:], cosT[:])
                nc.sync.dma_start(sin_sb[:], sinS[:])
                wq_sb = [xw.tile([P, 512], f32r, tag=f"wq{i}", name=f"wqs{i}")
                         for i in range(8)]
                wk_sb = [xw.tile([P, 512], f32r, tag=f"wk{i}", name=f"wks{i}")
                         for i in range(8)]
                wv_sb = [xw.tile([P, 512], f32r, tag=f"wv{i}", name=f"wvs{i}")
                         for i in range(8)]
                for i in range(8):
                    nc.sync.dma_start(xt[i][:], xT[i * P:(i + 1) * P, :])
                    nc.sync.dma_start(wv_sb[i][:], wvT[i * P:(i + 1) * P, :])
                    nc.sync.dma_start(wq_sb[i][:], wqT[i * P:(i + 1) * P, :])
                    nc.sync.dma_start(wk_sb[i][:], wkT[i * P:(i + 1) * P, :])

                def project_v(st):
                    ps = pps.tile([P, 512], f32, tag="ps", name="ps")
                    for kd in range(8):
                        nc.tensor.matmul(
                            ps[:],
                            lhsT=xt[kd][:, st * P:(st + 1) * P],
                            rhs=wv_sb[kd][:],
                            start=(kd == 0), stop=(kd == 7))
                    va3 = va[st][:].rearrange("p (h e) -> p h e", h=8, e=65)
                    nc.any.tensor_copy(
                        va3[:, :, 0:64],
                        ps[:].rearrange("p (h e) -> p h e", h=8, e=64))
                    nc.any.tensor_copy(
                        va3[:, :, 64:65],
                        ones_f32[:].rearrange("p (h o) -> p h o", o=1))

                def project_qk(mt, nch):
                    n0 = nch * 512
                    for wsb, dstT in ((wq_sb, qT), (wk_sb, kT)):
                        ps = pps.tile([P, 512], f32, tag="ps", name="ps")
                        for kd in range(8):
                            nc.tensor.matmul(
                                ps[:],
                                lhsT=wsb[kd][:, mt * P:(mt + 1) * P],
                                rhs=xt[kd][:, n0:n0 + 512],
                                start=(kd == 0), stop=(kd == 7))
                        dst = dstT[mt][:, n0:n0 + 512]
                        nc.vector.tensor_tensor(
                            dst, ps[:], cos_sb[:, n0:n0 + 512], Alu.mult)
                        tmp = rtmp.tile([P, 512], f32, tag="rt", name="rt")
                        nc.vector.stream_shuffle(tmp[:], ps[:], SWAP16)
                        nc.vector.tensor_tensor(
                            tmp[:], tmp[:], sin_sb[:, n0:n0 + 512], Alu.mult)
                        nc.vector.tensor_tensor(dst, dst, tmp[:], Alu.add)

                # qc0 needs q/k cols 0:512 (nch0) + va; emit those first
                for mt in range(4):
                    project_qk(mt, 0)
                    project_v(2 * mt)
                    project_v(2 * mt + 1)
                for mt in range(4):
                    project_qk(mt, 1)

            # ---- phase 3: differential attention + out-projection ----
            with (
                tc.tile_pool(name="pv_ps", bufs=4, space="PSUM") as pvp,
                tc.tile_pool(name="st_ps", bufs=4, space="PSUM") as stp,
                tc.tile_pool(name="epool", bufs=5) as epool,
                tc.tile_pool(name="post", bufs=2) as post,
                tc.tile_pool(name="small", bufs=1) as small,
                tc.tile_pool(name="po_sb", bufs=4) as posb,
            ):
                for qc in range(2):
                    q0 = qc * 512
                    kts = _kts(qc)
                    last_kt = kts[-1][0]
                    deferred = []
                    for pt in range(4):
                        pvs = [pvp.tile([P, 512], f32, tag="pv", name="pv")
                               for _ in range(4)]
                        for kt, off in kts:
                            j0 = 0 if off is None else off
                            sts = []
                            for gq in range(4):
                                st_ps = stp.tile([P, 512], f32, tag="st",
                                                 name="st")
                                nc.tensor.matmul(
                                    st_ps[:, j0:],
                                    lhsT=kT[pt][gq * 32:(gq + 1) * 32,
                                                kt * P:(kt + 1) * P],
                                    rhs=qT[pt][gq * 32:(gq + 1) * 32,
                                               q0 + j0:q0 + 512],
                                    start=True, stop=True,
                                    tile_position=(gq * 32, 0))
                                sts.append(st_ps)
                            for gq in range(4):
                                e = epool.tile([P, 512], f32r, tag="e",
                                               name="e")
                                nc.scalar.activation(
                                    e[:, j0:], sts[gq][:, j0:], Act.Exp,
                                    scale=SCALE)
                                if off is not None:
                                    nc.vector.tensor_tensor(
                                        e[:, j0:j0 + P], e[:, j0:j0 + P],
                                        md_sb[:], Alu.mult)
                                h_loc = (pt * P + gq * 32) // 64
                                nc.tensor.matmul(
                                    pvs[gq][0:65, j0:],
                                    lhsT=va[kt][:, h_loc * 65:(h_loc + 1) * 65],
                                    rhs=e[:, j0:],
                                    start=(kt == 0), stop=(kt == last_kt))

                        # free the PV banks ASAP: stage A and Z to SBUF
                        a1 = post.tile([P, 512], f32, tag="a1")
                        a2 = post.tile([P, 512], f32, tag="a2")
                        zs = [small.tile([1, 512], f32, tag=f"z{i}",
                                         name=f"z{i}", bufs=4) for i in range(4)]
                        for hp in range(2):
                            nc.vector.tensor_copy(a1[hp * 64:(hp + 1) * 64, :],
                                                  pvs[2 * hp][0:64, :])
                            nc.vector.tensor_copy(a2[hp * 64:(hp + 1) * 64, :],
                                                  pvs[2 * hp + 1][0:64, :])
                            nc.vector.tensor_copy(zs[2 * hp][:],
                                                  pvs[2 * hp][64:65, :])
                            nc.vector.tensor_copy(zs[2 * hp + 1][:],
                                                  pvs[2 * hp + 1][64:65, :])
                        bc1 = post.tile([P, 512], f32, tag="bc1")
                        bc2 = post.tile([P, 512], f32, tag="bc2")
                        ubc = post.tile([64, 512], f32, tag="ubc")
                        ubc2 = post.tile([64, 512], f32, tag="ubc2")
                        nc.gpsimd.partition_broadcast(bc1[0:64, :], zs[0][:])
                        nc.gpsimd.partition_broadcast(ubc[:], zs[2][:])
                        nc.vector.tensor_copy(bc1[64:128, :], ubc[:])
                        nc.gpsimd.partition_broadcast(bc2[0:64, :], zs[1][:])
                        nc.gpsimd.partition_broadcast(ubc2[:], zs[3][:])
                        nc.vector.tensor_copy(bc2[64:128, :], ubc2[:])
                        t1 = post.tile([P, 512], f32, tag="t1")
                        t2 = post.tile([P, 512], f32, tag="t2")
                        nc.vector.tensor_tensor(t1[:], a1[:], bc2[:], Alu.mult)
                        nc.vector.tensor_tensor(t2[:], a2[:], bc1[:], Alu.mult)
                        negw = post.tile([P, 512], f32, tag="negw", bufs=4)
                        nc.vector.scalar_tensor_tensor(
                            negw[:], in0=t2[:], scalar=lam_sb[:, 0:1],
                            in1=t1[:], op0=Alu.mult, op1=Alu.subtract)
                        sq = post.tile([P, 512], f32r, tag="sq", bufs=4)
                        nc.vector.tensor_tensor(sq[:], negw[:], negw[:], Alu.mult)
                        deferred.append((pt, zs, negw, sq))

                    # deferred normalization: PE ms-matmuls land after all
                    # four pt blocks so the in-order PE stream never stalls
                    for pt, zs, negw, sq in deferred:
                        ms_ps = stp.tile([P, 512], f32, tag="st", name="msps")
                        nc.tensor.matmul(ms_ps[:], lhsT=hs_sb[:], rhs=sq[:],
                                         start=True, stop=True)
                        bcm = post.tile([P, 512], f32, tag="bcm")
                        ubm = post.tile([64, 512], f32, tag="ubm")
                        for hp in range(2):
                            r0 = hp * 64
                            z12 = small.tile([1, 512], f32, tag="z12", bufs=2)
                            nc.vector.tensor_tensor(
                                z12[:], zs[2 * hp][:], zs[2 * hp + 1][:],
                                Alu.mult)
                            msb = small.tile([1, 512], f32, tag="msb", bufs=2)
                            nc.vector.scalar_tensor_tensor(
                                msb[:], in0=z12[:], scalar=EPS, in1=z12[:],
                                op0=Alu.mult, op1=Alu.mult)
                            msb2 = small.tile([1, 512], f32, tag="msb2", bufs=2)
                            nc.vector.scalar_tensor_tensor(
                                msb2[:], in0=ms_ps[r0:r0 + 1, :],
                                scalar=1.0 / 64.0, in1=msb[:],
                                op0=Alu.mult, op1=Alu.add)
                            if hp == 0:
                                nc.gpsimd.partition_broadcast(bcm[0:64, :],
                                                              msb2[:])
                            else:
                                nc.gpsimd.partition_broadcast(ubm[:], msb2[:])
                                nc.vector.tensor_copy(bcm[64:128, :], ubm[:])
                        srt = post.tile([P, 512], f32, tag="srt")
                        nc.scalar.activation(srt[:], bcm[:], Act.Sqrt)
                        rstd = post.tile([P, 512], f32, tag="rstd")
                        nc.vector.reciprocal_approx_fast(rstd[:], srt[:])
                        nc.vector.scalar_tensor_tensor(
                            attnT[pt][:, q0:q0 + 512],
                            in0=negw[:], scalar=-(1.0 - LAMBDA_INIT),
                            in1=rstd[:], op0=Alu.mult, op1=Alu.mult)

                    # out-projection for this seq half; on the second half,
                    # interleave the two reduce-scatter chunks
                    for mo in range(8):
                        ps = stp.tile([P, 512], f32, tag="st", name="wops")
                        for kc in range(4):
                            nc.tensor.matmul(
                                ps[:],
                                lhsT=wo_sb[kc][:, mo * P:(mo + 1) * P],
                                rhs=attnT[kc][:, q0:q0 + 512],
                                start=(kc == 0), stop=(kc == 3))
                        po = posb.tile([P, 512], bf16, tag="po")
                        nc.any.tensor_copy(po[:], ps[:])
                        nc.sync.dma_start(
                            po_dram[mo // 4, qc,
                                    (mo % 4) * P:(mo % 4 + 1) * P, :], po[:])
                        if qc == 1 and mo == 3:
                            nc.gpsimd.collective_compute(
                                "ReduceScatter",
                                mybir.AluOpType.add,
                                replica_groups=[[0, 1], [2, 3], [4, 5], [6, 7]],
                                ins=[po_dram[0].opt()],
                                outs=[rs_dram[0].opt()],
                            )
                    nc.gpsimd.collective_compute(
                        "ReduceScatter",
                        mybir.AluOpType.add,
                        replica_groups=[[0, 1], [2, 3], [4, 5], [6, 7]],
                        ins=[po_dram[1].opt()],
                        outs=[rs_dram[1].opt()],
                    ) if qc == 1 else None

            _stk.close()

            # ---- phase 5: FFN + residual + final RMS on seq shard ----
            with (
                tc.tile_pool(name="aT", bufs=1) as atp,
                tc.tile_pool(name="astage", bufs=2) as astage,
                tc.tile_pool(name="h1", bufs=1) as h1p,
                tc.tile_pool(name="w1p", bufs=6) as w1p,
                tc.tile_pool(name="w2p", bufs=3) as w2p,
                tc.tile_pool(name="yT", bufs=1) as ytp,
                tc.tile_pool(name="fin", bufs=2) as finp,
                tc.tile_pool(name="sm2", bufs=1) as sm2,
            ):
                aTr = [atp.tile([P, 512], f32r, tag=f"ar{i}", name=f"ar{i}")
                       for i in range(8)]
                for i in range(8):
                    stg = astage.tile([P, 512], bf16, tag="stg")
                    nc.sync.dma_start(
                        stg[:], rs_dram[i // 4, (i % 4) * P:(i % 4 + 1) * P, :])
                    nc.vector.tensor_copy(aTr[i][:], stg[:])
                    if debug:
                        nc.gpsimd.dma_start(dbg_rs[i * P:(i + 1) * P, :], stg[:])
                if debug:
                    for dh in range(2):
                        for r in range(2):
                            nc.gpsimd.dma_start(
                                dbg_po[r * D + dh * 512:r * D + (dh + 1) * 512, :],
                                po_dram[dh, r])

                h1 = [h1p.tile([P, 512], f32r, tag=f"h1_{i}", name=f"h1_{i}")
                      for i in range(32)]
                with tc.tile_pool(name="h1_ps", bufs=4, space="PSUM") as h1ps:
                    for mf in range(32):
                        wt = w1p.tile([P, 1024], f32r, tag="w1t", name="w1t")
                        nc.sync.dma_start(wt[:], w1s[mf, :, :])
                        ps = h1ps.tile([P, 512], f32, tag="h1ps", name="h1ps")
                        for kd in range(8):
                            nc.tensor.matmul(
                                ps[:], lhsT=wt[:, kd * P:(kd + 1) * P],
                                rhs=aTr[kd][:], start=(kd == 0), stop=(kd == 7))
                        nc.scalar.activation(h1[mf][:], ps[:], Act.Relu,
                                             bias=b1_sb[:, mf:mf + 1])

                # h2: 8 persistent PSUM accumulators, stream w2 tiles
                with tc.tile_pool(name="h2_ps", bufs=1, space="PSUM") as h2ps:
                    ps8 = [h2ps.tile([P, 512], f32, tag=f"h2_{mo}",
                                     name=f"h2_{mo}") for mo in range(8)]
                    for kf in range(32):
                        wt2 = w2p.tile([P, 1024], f32r, tag="w2t", name="w2t")
                        nc.sync.dma_start(wt2[:], w2T[kf * P:(kf + 1) * P, :])
                        for mo in range(8):
                            nc.tensor.matmul(
                                ps8[mo][:], lhsT=wt2[:, mo * P:(mo + 1) * P],
                                rhs=h1[kf][:], start=(kf == 0), stop=(kf == 31))
                    yt = [ytp.tile([P, 512], f32, tag=f"y{i}", name=f"y{i}")
                          for i in range(8)]
                    for mo in range(8):
                        nc.vector.scalar_tensor_tensor(
                            yt[mo][:], in0=ps8[mo][:],
                            scalar=b2_sb[:, mo:mo + 1], in1=aTr[mo][:],
                            op0=Alu.add, op1=Alu.add)

                # final RMS over D (partition dim across the 8 tiles)
                with tc.tile_pool(name="rms_ps", bufs=1, space="PSUM") as rmsps:
                    ms_ps = rmsps.tile([P, 512], f32, tag="rmsps", name="rmsps")
                    for mo in range(8):
                        sq = finp.tile([P, 512], f32r, tag="fsq", name="fsq")
                        nc.vector.tensor_tensor(sq[:], yt[mo][:], yt[mo][:],
                                                Alu.mult)
                        nc.tensor.matmul(ms_ps[0:1, :], lhsT=ones_r[:],
                                         rhs=sq[:], start=(mo == 0),
                                         stop=(mo == 7))
                    srt = sm2.tile([1, 512], f32, tag="fsrt")
                    nc.scalar.activation(srt[:], ms_ps[0:1, :], Act.Sqrt,
                                         scale=1.0 / 1024.0, bias=eps_sb[:])
                    rstd = sm2.tile([1, 512], f32, tag="frstd")
                    nc.vector.reciprocal_approx_fast(rstd[:], srt[:])
                    bcr = sm2.tile([P, 512], f32, tag="fbcr")
                    nc.gpsimd.partition_broadcast(bcr[:], rstd[:])
                    for mo in range(8):
                        ot = finp.tile([P, 512], f32, tag="fot", name="fot")
                        nc.vector.scalar_tensor_tensor(
                            ot[:], in0=yt[mo][:], scalar=rw_sb[:, mo:mo + 1],
                            in1=bcr[:], op0=Alu.mult, op1=Alu.mult)
                        nc.sync.dma_start(outT[mo * P:(mo + 1) * P, :], ot[:])

    nc.compile()
    return nc


def _host_prep(inputs):
    x = np.ascontiguousarray(np.asarray(inputs["x"], dtype=np.float32))
    Wq = np.asarray(inputs["Wq"], dtype=np.float32)
    Wk = np.asarray(inputs["Wk"], dtype=np.float32)
    Wv = np.asarray(inputs["Wv"], dtype=np.float32)
    Wo = np.asarray(inputs["Wo"], dtype=np.float32)
    W1 = np.asarray(inputs["W1"], dtype=np.float32)
    b1 = np.asarray(inputs["b1"], dtype=np.float32)
    W2 = np.asarray(inputs["W2"], dtype=np.float32)
    b2 = np.asarray(inputs["b2"], dtype=np.float32)
    rmsw = np.asarray(inputs["rms_weight"], dtype=np.float32)
    lam = float(np.exp(np.dot(np.asarray(inputs["lambda_q1"], np.float64),
                              np.asarray(inputs["lambda_k1"], np.float64)))
                - np.exp(np.dot(np.asarray(inputs["lambda_q2"], np.float64),
                                np.asarray(inputs["lambda_k2"], np.float64)))
                + LAMBDA_INIT)

    half = HD // 2
    cos16 = sin16 = None
    try:
        import jax.numpy as jnp
        freqs = 1.0 / (10000.0 ** (jnp.arange(half, dtype=jnp.float32) / half))
        ang = jnp.arange(S, dtype=jnp.float32)[:, None] * freqs[None, :]
        cos16 = np.asarray(jnp.cos(ang)).T.astype(np.float32)
        sin16 = np.asarray(jnp.sin(ang)).T.astype(np.float32)
    except Exception:
        pass
    if cos16 is None:
        freqs = (1.0 / (10000.0 ** (np.arange(half, dtype=np.float32)
                                    / np.float32(half)))).astype(np.float32)
        ang = (np.arange(S, dtype=np.float32)[:, None] * freqs[None, :])
        cos16 = np.cos(ang.astype(np.float32)).T.astype(np.float32)
        sin16 = np.sin(ang.astype(np.float32)).T.astype(np.float32)

    cosT = np.ascontiguousarray(np.tile(np.concatenate([cos16, cos16], 0), (4, 1)))
    sinS = np.ascontiguousarray(
        np.tile(np.concatenate([-sin16, sin16], 0), (4, 1))).astype(np.float32)
    perm32 = np.concatenate([np.arange(0, 32, 2), np.arange(1, 32, 2)])

    hsel = np.zeros((128, 128), np.float32)
    hsel[0:64, 0] = 1.0
    hsel[64:128, 64] = 1.0
    mdiag = (np.arange(128)[:, None] <= np.arange(128)[None, :]).astype(np.float32)

    b1c = np.ascontiguousarray(b1.reshape(32, 128).T)
    b2c = np.ascontiguousarray(b2.reshape(8, 128).T)
    rmswc = np.ascontiguousarray(rmsw.reshape(8, 128).T)
    lam128 = np.full((128, 1), lam, np.float32)
    # w1s[mf][p, kd*128+j] = W1.T[kd*128+p, mf*128+j]
    w1s = np.ascontiguousarray(
        W1.T.reshape(8, 128, 32, 128).transpose(2, 1, 0, 3).reshape(32, 128, 1024))
    w2T = np.ascontiguousarray(W2.T)

    in_maps = []
    for c in range(NCORES):
        b, g = c // 2, c % 2
        chans = np.arange(g * 512, (g + 1) * 512)
        permed = np.concatenate(
            [c0 * 32 + perm32 for c0 in range(g * 16, (g + 1) * 16)])
        in_maps.append({
            "xT": np.ascontiguousarray(x[b].T),
            "wqT": np.ascontiguousarray(Wq[permed, :].T),
            "wkT": np.ascontiguousarray(Wk[permed, :].T),
            "wvT": np.ascontiguousarray(Wv[chans, :].T),
            "woT": np.ascontiguousarray(Wo[:, chans].T),
            "w1s": w1s, "w2T": w2T,
            "b1c": b1c, "b2c": b2c, "rmswc": rmswc, "lam128": lam128,
            "cosT": cosT, "sinS": sinS,
            "hsel": hsel, "mdiag": mdiag,
        })
    return in_maps


def kernel(**inputs):
    global LAST_RESULT
    from concourse.bass_utils import run_bass_kernel_spmd

    key = "nc_dbg" if os.environ.get("KERNEL_DEBUG", "0") == "1" else "nc"
    if key not in _PROGRAM:
        _PROGRAM[key] = _build_program()
    nc = _PROGRAM[key]

    in_maps = _host_prep(inputs)
    trace = bool(int(os.environ.get("KERNEL_TRACE", "0")))
    res = run_bass_kernel_spmd(nc, in_maps, list(range(NCORES)), trace=trace)
    LAST_RESULT = res

    out = np.empty((B, S, D), np.float32)
    for c in range(NCORES):
        b, g = c // 2, c % 2
        out[b, g * 512:(g + 1) * 512, :] = res.results[c]["outT"].T
    return out
